# revision 17
# baseline (speedup 1.0000x reference)
"""DeepSAT GNN message-passing kernel for 8 Trainium2 NeuronCores.

Algorithm notes (validated numerically against the reference):
  - Every node is updated exactly once, at step l = forward_level (levels
    1..19; level-0 nodes keep h0 forever). At update time the node's own
    hidden state is still h0, so the GRU "hidden side" gates are constant
    vectors computable on the host.
  - Nodes with deg==0 (no in-edges) and level>=1 get msg=0, hence a single
    constant h_z = GRU(0, h0); their prediction MLP(h_z) is a host-side
    constant, as is MLP(h0) for level-0 nodes. Both node classes are
    excluded from the device rank space entirely; edges sourced at them
    enter the seed counts (n0 for h0-sources, nz for h_z-sources).
  - msg_i = W @ (S_i + n0_i*h0 + nz_i*h_z) + deg_i*b. With u = W^-1 b this
    folds to msg_i = W @ S'_i, S'_i = S_i + n0_i*h0 + nz_i*h_z + deg_i*u,
    so the per-gate input is gi_g = (wih_g @ W) @ S'_i + bih_g.
  - The GRU output is written as h = h0 + t4, t4 = sigm(-z)*(tanh(n) - h0).
    Only t4 is produced on-engine; +h0 is folded into the post-transpose
    copy (row broadcast) and W1@h0 into the MLP's first bias.
  - Everything flows in fp16 (fp32 PSUM accumulation): 4x matmul
    throughput vs fp32, half the AllGather/gather bytes, and fp16's
    11-bit mantissa keeps the end-to-end relative error at ~7e-4.
  - Gathers use the SWDGE dma_gather instruction: ONE instruction per
    (level, phase, 32k-source-segment) regardless of edge count, versus
    ~1us of fixed descriptor-generation overhead per 128-row indirect DMA.

Device schedule per level l (SPMD on 8 cores):
  gather h[src] for this level's "fresh" edges (src level == l-1),
  accumulate via one-hot matmuls into PSUM (seeded with the n0/nz/deg
  terms), fused GRU producing t4, PE-transpose, +h0, DMA to the AllGather
  input, AllGather into the replicated h_store, then (overlapping the
  collective) the MLP head for this level plus the next level's "old"
  edges (src level < l), whose gather only reads rows below this level's
  slab.
"""

import sys

import numpy as np

sys.path.insert(0, "/opt/trn_rl_repo")

P = 128
D = 128
NC = 8
GW = 512          # psum group width (one bank of fp32)
SEG = 32768       # dma_gather int16 index window (rows)

_COMPILED = {}


# ---------------------------------------------------------------------------
# Host-side math helpers
# ---------------------------------------------------------------------------

def _sigmoid(x):
    return 1.0 / (1.0 + np.exp(-x))


def _prep_weights(inp):
    f64 = np.float64
    W = inp["aggr_w"].astype(f64)
    b = inp["aggr_b"].astype(f64)
    h0 = (inp["emd_w"][:, 0] + inp["emd_b"]).astype(f64)
    wih = inp["gru_wih"].astype(f64)
    whh = inp["gru_whh"].astype(f64)
    bih = inp["gru_bih"].astype(f64)
    bhh = inp["gru_bhh"].astype(f64)
    u = np.linalg.solve(W, b)
    assert np.abs(W @ u - b).max() < 1e-5

    ghc = whh @ h0 + bhh
    hr_c, hz_c, hn_c = ghc[:D], ghc[D:2 * D], ghc[2 * D:]
    bih_r, bih_z, bih_n = bih[:D], bih[D:2 * D], bih[2 * D:]
    WgT = [(wih[g * D:(g + 1) * D] @ W).T for g in range(3)]

    # constant hidden state of zero-in-degree nodes (msg = 0)
    r = _sigmoid(bih_r + hr_c)
    z = _sigmoid(bih_z + hz_c)
    n = np.tanh(bih_n + r * hn_c)
    h_z = (1.0 - z) * n + z * h0

    W1 = inp["w1"].astype(f64)   # [256, 128]
    b1 = inp["b1"].astype(f64)
    W2 = inp["w2"].astype(f64)   # [256, 256]
    b2 = inp["b2"].astype(f64)
    w3 = inp["w3"].astype(f64)   # [1, 256]
    b3 = inp["b3"].astype(f64)

    def mlp(h):
        z1 = np.maximum(W1 @ h + b1, 0.0)
        z2 = np.maximum(W2 @ z1 + b2, 0.0)
        return float(w3[0] @ z2 + b3[0])

    pred0 = mlp(h0)
    predz = mlp(h_z)

    b1p = b1 + W1 @ h0          # W1@h0 folded into the first MLP bias

    bf16 = np.float16

    wblocks = [
        WgT[0], WgT[1], WgT[2], np.diag(hn_c),
        W1[0:128, :].T, W1[128:256, :].T,
        W2[0:128, 0:128].T, W2[0:128, 128:256].T,
        W2[128:256, 0:128].T, W2[128:256, 128:256].T,
        np.eye(128),
    ]
    wmat = np.concatenate(wblocks, axis=1).astype(bf16)   # [128, 11*128]

    vcols = np.stack([
        h0,                      # 0: h0 column (tensor_scalar operand)
        bih_r + hr_c,            # 1: sigmoid bias for r
        -(bih_z + hz_c),         # 2: sigmoid bias for z' (scale = -1)
        bih_n,                   # 3: tanh bias for n
        b1p[0:128],              # 4
        b1p[128:256],            # 5
        b2[0:128],               # 6
        b2[128:256],             # 7
        np.full(128, b3[0]),     # 8: b3 (row 0 used)
    ], axis=1).astype(np.float32)                         # [128, 9]

    vcolsb = np.stack([w3[0, 0:128], w3[0, 128:256]],
                      axis=1).astype(bf16)                # [128, 2]

    rowc = np.zeros((1, 1024), np.float32)
    rowc[0, 0:512] = np.arange(512, dtype=np.float32)     # iota for one-hots
    rowc[0, 512:1024] = np.tile(h0.astype(np.float32), 4)  # +h0 after transpose
    rowc = np.repeat(rowc, 128, axis=0)                   # full-partition tile

    vr3 = np.stack([h0, u, h_z], axis=0).astype(bf16)     # [3, 128] seed lhsT

    return {
        "wmat": wmat, "vcols": vcols, "vcolsb": vcolsb, "rowc": rowc,
        "vr3": vr3, "pred0": pred0, "predz": predz, "bf16": bf16,
    }


WM = {name: i for i, name in enumerate(
    ["WgT_r", "WgT_z", "WgT_n", "diag_hn", "W1Ta", "W1Tb",
     "W2_k0m0", "W2_k1m0", "W2_k0m1", "W2_k1m1", "ident"])}
VC = {name: i for i, name in enumerate(
    ["h0", "bias_r", "nbias_z", "bias_n", "b1pa", "b1pb", "b2a", "b2b",
     "b3"])}


# ---------------------------------------------------------------------------
# Host-side preprocessing: rank space + packed edge tables
# ---------------------------------------------------------------------------

def _preprocess(forward_level, edge_index, num_levels):
    fl = np.asarray(forward_level).astype(np.int64)
    ei = np.asarray(edge_index).astype(np.int64)
    src, dst = ei[0], ei[1]
    N = fl.shape[0]
    NL = num_levels

    deg = np.bincount(dst, minlength=N).astype(np.int64)
    lv_s, lv_d = fl[src], fl[dst]
    constn = (deg == 0) & (fl >= 1)      # h == h_z forever
    lvl0 = fl == 0                       # h == h0 forever
    nonc = ~(constn | lvl0)              # device rank space

    act = (lv_s >= 1) & (lv_s < lv_d)
    gact = act & ~constn[src]            # gathered edges
    zact = act & constn[src]             # const-source edges -> nz count
    nz = np.bincount(dst[zact], minlength=N).astype(np.int64)
    ngact = np.bincount(dst[gact], minlength=N).astype(np.int64)
    n0 = deg - nz - ngact

    is_src = np.zeros(N, bool)
    is_src[src[gact]] = True

    # --- rank space: nonconst nodes sorted by level, levels 1..NL-1 ---
    n_l = np.bincount(fl[nonc], minlength=NL).astype(np.int64)
    n_l[0] = 0
    # +2*NC slack so the per-core interleaved source/non-source split fits
    pad_l = ((n_l + 2 * NC + NC * P - 1) // (NC * P)) * (NC * P)
    pad_l = np.maximum(pad_l, NC * P)
    pad_l[0] = 0
    L_off = np.zeros(NL + 1, np.int64)
    L_off[1:] = np.cumsum(pad_l)
    Vc = (pad_l // NC).astype(np.int64)
    Voff = np.zeros(NL + 1, np.int64)
    Voff[1:] = np.cumsum(Vc)
    nblk = (Vc // P).astype(np.int64)
    sumVc = int(Voff[NL])
    NpadTot = int(L_off[NL])

    # per (level, core): gather-source nodes first, so the AllGather only
    # needs to move the first Kpad rows of each core's shard
    rank = np.full(N, -1, np.int64)
    rank2 = np.full(N, -1, np.int64)     # rank in the compacted h_store
    Kpad = np.zeros(NL, np.int64)
    for l in range(1, NL):
        lv_nodes = np.where(nonc & (fl == l))[0]
        s_nodes = lv_nodes[is_src[lv_nodes]]
        r_nodes = lv_nodes[~is_src[lv_nodes]]
        S, R = len(s_nodes), len(r_nodes)
        s_core = np.arange(S, dtype=np.int64) % NC
        s_pos = np.arange(S, dtype=np.int64) // NC
        Kc = np.bincount(s_core, minlength=NC).astype(np.int64)
        r_core = np.arange(R, dtype=np.int64) % NC
        r_pos = Kc[r_core] + np.arange(R, dtype=np.int64) // NC
        assert R == 0 or r_pos.max() < int(Vc[l])
        rank[s_nodes] = L_off[l] + s_core * Vc[l] + s_pos
        rank[r_nodes] = L_off[l] + r_core * Vc[l] + r_pos
        if S > 0:
            Kpad[l] = ((int(Kc.max()) + P - 1) // P) * P
            rank2[s_nodes] = s_core * Kpad[l] + s_pos  # + L2_off[l] below
    L2_off = np.zeros(NL + 1, np.int64)
    L2_off[1:] = np.cumsum(NC * Kpad)
    NpadTot2 = int(L2_off[NL])
    sn = np.where(rank2 >= 0)[0]
    rank2[sn] += L2_off[fl[sn]]
    node_of_rank = np.full(NpadTot, -1, np.int64)
    nodes = np.where(nonc)[0]
    node_of_rank[rank[nodes]] = nodes

    # --- per-core seed counts [NC, 3, sumVc] (rows: n0, deg, nz) ---
    cnts = np.zeros((NC, 3, sumVc), np.float32)
    for c in range(NC):
        grs = []
        for l in range(NL):
            grs.append(L_off[l] + c * Vc[l] + np.arange(Vc[l]))
        gr = np.concatenate(grs)
        nd = node_of_rank[gr]
        m = nd >= 0
        cnts[c, 0, m] = n0[nd[m]]
        cnts[c, 1, m] = deg[nd[m]]
        cnts[c, 2, m] = nz[nd[m]]

    # --- packed edge tables ---
    er = np.where(gact)[0]
    e_lvl = lv_d[er]
    e_srcrank = rank2[src[er]]
    assert e_srcrank.min(initial=0) >= 0
    e_dstrank = rank[dst[er]]
    e_local = e_dstrank - L_off[e_lvl]
    e_core = e_local // Vc[e_lvl]
    e_wl = e_local % Vc[e_lvl]
    e_fresh = lv_s[er] == (e_lvl - 1)

    idx_cols = [[] for _ in range(NC)]   # int16 [128, k] blocks
    rnk_cols = [[] for _ in range(NC)]   # f32 [128] columns
    icol = [0]
    rcol = [0]

    def pack_gather(base, sel_core_edges, nch):
        """Append idx columns for one dma_gather; returns icol0."""
        i0 = icol[0]
        for c in range(NC):
            es = sel_core_edges[c]
            vals = np.zeros(nch * P, np.int64)
            vals[:len(es)] = e_srcrank[es] - base
            assert vals.max(initial=0) < SEG and vals.min(initial=0) >= 0
            blk = np.zeros((128, nch * 8), np.int16)
            # HW (queue 0) reads index i at partition 16 + i%16, col i//16;
            # the interpreter reads partitions 0..15 — fill both.
            blk[:16, :] = vals.astype(np.int16).reshape(nch * 8, 16).T
            blk[16:32, :] = blk[:16, :]
            idx_cols[c].append(blk)
        icol[0] += nch * 8
        return i0

    def pack_phase(l, sel, is_fresh):
        """Pack one (level, phase); returns phase dict or None."""
        percore_all = [er_idx[sel & (e_core == c)] for c in range(NC)]
        if all(len(x) == 0 for x in percore_all):
            return None
        if is_fresh:
            # fresh: single segment based at the previous level's compacted
            # slab, which spans [L2_off[l-1], L2_off[l])
            bases = [int(L2_off[l - 1])]
            bound = int(L2_off[l])
        else:
            segs = sorted(set((e_srcrank[sel] // SEG).tolist()))
            bases = [int(s * SEG) for s in segs]
            bound = int(L2_off[l - 1])
        gathers = []
        incids = []
        ch0 = 0
        for base in bases:
            rows = min(SEG, bound - base)
            sel_b = sel & (e_srcrank >= base) & (e_srcrank < base + SEG)
            percore = []
            for c in range(NC):
                es = er_idx[sel_b & (e_core == c)]
                es = es[np.argsort(e_wl[es], kind="stable")]
                percore.append(es)
            cnt = max(len(x) for x in percore)
            if cnt == 0:
                continue
            nch = (cnt + P - 1) // P
            i0 = pack_gather(base, percore, nch)
            for j in range(nch):
                groups = set()
                chunk_es = [x[j * P:(j + 1) * P] for x in percore]
                for es in chunk_es:
                    groups.update((e_wl[es] // GW).tolist())
                for g in sorted(groups):
                    for c in range(NC):
                        es = chunk_es[c]
                        rv = np.full(P, -1.0, np.float32)
                        ing = (e_wl[es] // GW) == g
                        rv[np.where(ing)[0]] = (e_wl[es[ing]] - g * GW)
                        rnk_cols[c].append(rv)
                    incids.append([ch0 + j, int(g), rcol[0], False])
                    rcol[0] += 1
            gathers.append((base, rows, nch, i0, ch0))
            ch0 += nch
        if not gathers:
            return None
        return {"gathers": gathers, "incids": incids, "nch_total": ch0}

    er_idx = np.arange(len(er), dtype=np.int64)
    levels = [None] * NL
    for l in range(1, NL):
        in_lvl = e_lvl == l
        old = pack_phase(l, in_lvl & ~e_fresh, False)
        fresh = pack_phase(l, in_lvl & e_fresh, True)
        ngrp = (int(Vc[l]) + GW - 1) // GW
        # stop-flag per psum group: seeds -> old -> fresh (emission order)
        seed_stop = [True] * ngrp
        for ph in (old, fresh):
            if ph is None:
                continue
            for it in ph["incids"]:
                seed_stop[it[1]] = False
        lastmark = {}
        for name, ph in (("old", old), ("fresh", fresh)):
            if ph is None:
                continue
            for i, it in enumerate(ph["incids"]):
                lastmark[it[1]] = (name, i)
        for name, ph in (("old", old), ("fresh", fresh)):
            if ph is None:
                continue
            for i, it in enumerate(ph["incids"]):
                it[3] = lastmark.get(it[1]) == (name, i)
        levels[l] = {"old": old, "fresh": fresh, "seed_stop": seed_stop,
                     "ngrp": ngrp}

    TC = max(icol[0], 8)
    RC = max(rcol[0], 1)
    idxs = np.zeros((NC, 128, TC), np.int16)
    rnks = np.full((NC, 128, RC), -1.0, np.float32)
    for c in range(NC):
        if idx_cols[c]:
            idxs[c, :, :icol[0]] = np.concatenate(idx_cols[c], axis=1)
        if rnk_cols[c]:
            rnks[c, :, :rcol[0]] = np.stack(rnk_cols[c], axis=1)

    return {
        "N": N, "NL": NL, "n_l": n_l, "pad": pad_l, "L_off": L_off,
        "Vc": Vc, "Voff": Voff, "nblk": nblk, "sumVc": sumVc,
        "NpadTot": NpadTot, "TC": TC, "RC": RC,
        "Kpad": Kpad, "L2_off": L2_off, "NpadTot2": NpadTot2,
        "rank2": rank2,
        "levels": levels, "idxs": idxs, "rnks": rnks, "cnts": cnts,
        "node_of_rank": node_of_rank, "lvl0": lvl0, "constn": constn,
    }


# ---------------------------------------------------------------------------
# Bass program
# ---------------------------------------------------------------------------

def _build(sched):
    import concourse.bacc as bacc
    import concourse.tile as tile
    from concourse import bass, mybir

    f32 = mybir.dt.float32
    bf16 = mybir.dt.float16
    i16 = mybir.dt.int16
    AF = mybir.ActivationFunctionType
    OP = mybir.AluOpType
    NL = sched["NL"]
    L_off = sched["L_off"]
    Vc = sched["Vc"]
    Voff = sched["Voff"]
    pad = sched["pad"]
    TC = sched["TC"]
    RC = sched["RC"]
    sumVc = sched["sumVc"]
    NpadTot = sched["NpadTot"]
    Kpad = sched["Kpad"]
    L2_off = sched["L2_off"]
    NpadTot2 = sched["NpadTot2"]
    Vcmax = int(Vc.max())
    Kpadmax = max(int(Kpad.max()), P)
    RG = [list(range(NC))]

    nc = bacc.Bacc("TRN2", target_bir_lowering=False, debug=False,
                   enable_asserts=False, num_devices=NC)

    wmat_d = nc.dram_tensor("wmat", [P, P * len(WM)], bf16, kind="ExternalInput")
    vc_d = nc.dram_tensor("vcols", [P, len(VC)], f32, kind="ExternalInput")
    vcb_d = nc.dram_tensor("vcolsb", [P, 2], bf16, kind="ExternalInput")
    rowc_d = nc.dram_tensor("rowc", [P, 1024], f32, kind="ExternalInput")
    vr3_d = nc.dram_tensor("vr3", [3, P], bf16, kind="ExternalInput")
    cnt_d = nc.dram_tensor("cnts", [3, sumVc], bf16, kind="ExternalInput")
    idx_d = nc.dram_tensor("idxs", [P, TC], i16, kind="ExternalInput")
    rnk_d = nc.dram_tensor("rnks", [P, RC], f32, kind="ExternalInput")
    io16_d = nc.dram_tensor("iota16", [P, 1536], bf16, kind="ExternalInput")
    pred_d = nc.dram_tensor("pred", [sumVc], f32, kind="ExternalOutput")
    h_store = nc.dram_tensor("h_store", [NpadTot2, D], bf16, kind="Internal",
                             addr_space="Shared")
    ag_in = [nc.dram_tensor(f"ag_in{i}", [Kpadmax, D], bf16, kind="Internal")
             for i in range(2)]

    with tile.TileContext(nc) as tc:
        cpool = tc.alloc_tile_pool(name="const", bufs=1)
        spool = tc.alloc_tile_pool(name="sbuf", bufs=2)
        gpool = tc.alloc_tile_pool(name="gath", bufs=2)
        opool = tc.alloc_tile_pool(name="oh", bufs=2)
        hpool = tc.alloc_tile_pool(name="hnew", bufs=6)
        ppool = tc.alloc_tile_pool(name="psS", bufs=3, space="PSUM")
        qpool = tc.alloc_tile_pool(name="psG", bufs=3, space="PSUM")
        tpool = tc.alloc_tile_pool(name="psT", bufs=1, space="PSUM")
        rpool = tc.alloc_tile_pool(name="psP", bufs=1, space="PSUM")

        # ---- load constants ----
        wm = cpool.tile([P, P * len(WM)], bf16, tag="wm")
        nc.sync.dma_start(out=wm[:], in_=wmat_d[:])
        vc = cpool.tile([P, len(VC)], f32, tag="vc")
        nc.sync.dma_start(out=vc[:], in_=vc_d[:])
        vcb = cpool.tile([P, 2], bf16, tag="vcb")
        nc.sync.dma_start(out=vcb[:], in_=vcb_d[:])
        rowc = cpool.tile([P, 1024], f32, tag="rowc")
        nc.sync.dma_start(out=rowc[:], in_=rowc_d[:])
        vr3 = cpool.tile([3, P], bf16, tag="vr3")
        nc.sync.dma_start(out=vr3[:], in_=vr3_d[:])
        idxs = cpool.tile([P, TC], i16, tag="idxs")
        nc.sync.dma_start(out=idxs[:], in_=idx_d[:])
        rnks = cpool.tile([P, RC], f32, tag="rnks")
        nc.sync.dma_start(out=rnks[:], in_=rnk_d[:])
        io16 = cpool.tile([P, 1536], bf16, tag="io16")
        nc.sync.dma_start(out=io16[:], in_=io16_d[:])

        def wmb(name):
            return wm[:, WM[name] * P:(WM[name] + 1) * P]

        def vcc(name):
            return vc[:, VC[name]:VC[name] + 1]

        # ---- per-level state ----
        S_ps = [None] * NL
        Hg_old = [None] * (NL + 1)
        last_ag = [None]

        def grp_widths(l):
            ws = []
            v = int(Vc[l])
            while v > 0:
                ws.append(min(GW, v))
                v -= GW
            return ws

        def emit_gathers(info, which, bound_level):
            """One dma_gather per source segment; reads h_store below
            L_off[bound_level]."""
            ph = info[which]
            if ph is None:
                return None
            hg = gpool.tile([P, ph["nch_total"] * D], bf16, tag="hg_" + which)
            for (base, rows, nch, i0, ch0) in ph["gathers"]:
                gi = nc.gpsimd.dma_gather(
                    out_ap=hg[:, ch0 * D:(ch0 + nch) * D].rearrange(
                        "p (k d) -> p k d", d=D),
                    in_ap=h_store[base:base + rows, :],
                    idxs_ap=idxs[:, i0:i0 + nch * 8],
                    num_idxs=nch * P,
                    num_idxs_reg=nch * P,
                    elem_size=D,
                )
                # dynamic DRAM reads are not region-tracked by Tile's shadow
                # memory: pin the RAW edge vs the latest AllGather by hand
                if last_ag[0] is not None:
                    tile.add_dep_helper(gi.ins, last_ag[0].ins, sync=True,
                                        reason="gather reads AllGather output")
            return hg

        def emit_onehots(info, which, widths):
            ph = info[which]
            if ph is None:
                return None
            n = len(ph["incids"])
            oh = opool.tile([P, n * GW], bf16, tag="oh_" + which)
            for i, (ch, g, rc, stop) in enumerate(ph["incids"]):
                w = widths[g]
                nc.vector.tensor_tensor(
                    out=oh[:, i * GW:i * GW + w],
                    in0=rnks[:, rc:rc + 1].to_broadcast([P, w]),
                    in1=rowc[:, 0:w],
                    op=OP.is_equal,
                )
            return oh

        def emit_chunks(l, which, hg, oh):
            info = sched["levels"][l]
            ph = info[which]
            if ph is None:
                return
            widths = grp_widths(l)
            for i, (ch, g, rc, stop) in enumerate(ph["incids"]):
                w = widths[g]
                nc.tensor.matmul(
                    out=S_ps[l][g][:, :w],
                    lhsT=hg[:, ch * D:(ch + 1) * D],
                    rhs=oh[:, i * GW:i * GW + w],
                    start=False, stop=stop, skip_group_check=True)

        def emit_seeds(l):
            info = sched["levels"][l]
            tiles = []
            v = int(Vc[l])
            off = int(Voff[l])
            cm = spool.tile([3, Vcmax], bf16, tag="cm")
            nc.sync.dma_start(out=cm[0:3, :v], in_=cnt_d[0:3, off:off + v])
            for g, w in enumerate(grp_widths(l)):
                sp = ppool.tile([P, GW], f32, tag="S", space="PSUM")
                nc.tensor.matmul(
                    out=sp[:, :w], lhsT=vr3[0:3, :],
                    rhs=cm[0:3, g * GW:g * GW + w],
                    start=True, stop=info["seed_stop"][g],
                    skip_group_check=True)
                tiles.append(sp)
            S_ps[l] = tiles

        def emit_mlp(l, g, w, rhs_sb):
            """MLP head for one 512-group. rhs_sb is t4 (h - h0); the W1@h0
            part lives in the b1p bias."""
            z1s = []
            for half in ("a", "b"):
                zp = qpool.tile([P, GW], f32, tag="G", space="PSUM")
                nc.tensor.matmul(out=zp[:, :w], lhsT=wmb("W1T" + half),
                                 rhs=rhs_sb[:, :w], start=True, stop=True)
                zs = spool.tile([P, GW], bf16, tag="z1" + half)
                # relu(x + b1p) on the vector engine (balances ACT load)
                nc.vector.tensor_scalar(out=zs[:, :w], in0=zp[:, :w],
                                        scalar1=vcc("b1p" + half),
                                        scalar2=0.0,
                                        op0=OP.add, op1=OP.max)
                z1s.append(zs)
            z2s = []
            for mi, mh in enumerate(("m0", "m1")):
                zp = qpool.tile([P, GW], f32, tag="G", space="PSUM")
                nc.tensor.matmul(out=zp[:, :w], lhsT=wmb("W2_k0" + mh),
                                 rhs=z1s[0][:, :w], start=True, stop=False)
                nc.tensor.matmul(out=zp[:, :w], lhsT=wmb("W2_k1" + mh),
                                 rhs=z1s[1][:, :w], start=False, stop=True)
                zs = spool.tile([P, GW], bf16, tag="z2" + mh)
                nc.scalar.activation(out=zs[:, :w], in_=zp[:, :w], func=AF.Relu,
                                     bias=vcc("b2a" if mi == 0 else "b2b"))
                z2s.append(zs)
            pp = rpool.tile([1, GW], f32, tag="pred", space="PSUM")
            nc.tensor.matmul(out=pp[:, :w], lhsT=vcb[:, 0:1], rhs=z2s[0][:, :w],
                             start=True, stop=False)
            nc.tensor.matmul(out=pp[:, :w], lhsT=vcb[:, 1:2], rhs=z2s[1][:, :w],
                             start=False, stop=True)
            ps = spool.tile([1, GW], f32, tag="psb")
            nc.scalar.activation(out=ps[:, :w], in_=pp[:, :w], func=AF.Identity,
                                 bias=vc[0:1, VC["b3"]:VC["b3"] + 1])
            off = int(Voff[l]) + g * GW
            nc.sync.dma_start(out=pred_d[off:off + w], in_=ps[0:1, :w])

        # seeds for level 1 (no old/fresh edges possible at level 1)
        emit_seeds(1)

        # ================= levels 1..NL-1 =================
        for l in range(1, NL):
            info = sched["levels"][l]
            widths = grp_widths(l)

            # fresh gather + one-hots + scatter matmuls for this level
            hg_f = emit_gathers(info, "fresh", l)
            oh_f = emit_onehots(info, "fresh", grp_widths(l))
            emit_chunks(l, "fresh", hg_f, oh_f)

            # old gather for the NEXT level: sources are below L_off[l], so
            # the dma_gather runs before this level's AllGather on the queue
            if l + 1 < NL:
                ninfo = sched["levels"][l + 1]
                Hg_old[l + 1] = (
                    emit_gathers(ninfo, "old", l),
                    emit_onehots(ninfo, "old", grp_widths(l + 1)),
                )

            # GRU per group: t4 = sigm(-z) * (tanh(n) - h0);  h = h0 + t4
            t4g = []
            for g, w in enumerate(widths):
                ssb = spool.tile([P, GW], bf16, tag="Ssb")
                nc.vector.tensor_copy(out=ssb[:, :w], in_=S_ps[l][g][:, :w])

                gr = qpool.tile([P, GW], f32, tag="G", space="PSUM")
                nc.tensor.matmul(out=gr[:, :w], lhsT=wmb("WgT_r"),
                                 rhs=ssb[:, :w], start=True, stop=True)
                gz = qpool.tile([P, GW], f32, tag="G", space="PSUM")
                nc.tensor.matmul(out=gz[:, :w], lhsT=wmb("WgT_z"),
                                 rhs=ssb[:, :w], start=True, stop=True)
                gn = qpool.tile([P, GW], f32, tag="G", space="PSUM")
                nc.tensor.matmul(out=gn[:, :w], lhsT=wmb("WgT_n"),
                                 rhs=ssb[:, :w], start=True, stop=False)

                rsb = spool.tile([P, GW], bf16, tag="rsb")
                nc.scalar.activation(out=rsb[:, :w], in_=gr[:, :w],
                                     func=AF.Sigmoid, bias=vcc("bias_r"))
                zsb = spool.tile([P, GW], f32, tag="zsb")
                nc.scalar.activation(out=zsb[:, :w], in_=gz[:, :w],
                                     func=AF.Sigmoid, bias=vcc("nbias_z"),
                                     scale=-1.0)
                nc.tensor.matmul(out=gn[:, :w], lhsT=wmb("diag_hn"),
                                 rhs=rsb[:, :w], start=False, stop=True)
                nsb = spool.tile([P, GW], f32, tag="nsb")
                nc.scalar.activation(out=nsb[:, :w], in_=gn[:, :w],
                                     func=AF.Tanh, bias=vcc("bias_n"))

                t3 = spool.tile([P, GW], f32, tag="t3")
                nc.vector.tensor_scalar(out=t3[:, :w], in0=nsb[:, :w],
                                        scalar1=vcc("h0"), scalar2=None,
                                        op0=OP.subtract)
                t4 = hpool.tile([P, GW], bf16, tag="t4")
                nc.vector.tensor_tensor(out=t4[:, :w], in0=t3[:, :w],
                                        in1=zsb[:, :w], op=OP.mult)
                t4g.append(t4)

            # transpose+stage only the source prefix (first Kpad rows of
            # this core's shard), AllGather it into the compacted h_store
            # (skipped for the last level: nothing reads it)
            kp = int(Kpad[l])
            if l < NL - 1 and kp > 0:
                agt = ag_in[l % 2]
                for g, w in enumerate(widths):
                    ws = min(w, kp - g * GW)
                    if ws <= 0:
                        break
                    tp = tpool.tile([P, GW], bf16, tag="tp", space="PSUM")
                    nb = ws // P
                    for b in range(nb):
                        nc.tensor.transpose(
                            out=tp[:, b * P:(b + 1) * P],
                            in_=t4g[g][:, b * P:(b + 1) * P],
                            identity=wmb("ident"))
                    tps = spool.tile([P, GW], bf16, tag="tps")
                    nc.vector.tensor_tensor(
                        out=tps[:, :ws], in0=tp[:, :ws],
                        in1=rowc[:, 512:512 + ws],
                        op=OP.add)
                    for b in range(nb):
                        row = g * GW + b * P
                        nc.sync.dma_start(out=agt[row:row + P, :],
                                          in_=tps[:, b * P:(b + 1) * P])
                cc = nc.gpsimd.collective_compute(
                    "AllGather", bass.mybir.AluOpType.bypass,
                    replica_groups=RG,
                    ins=[agt[0:kp, :].opt()],
                    outs=[h_store[int(L2_off[l]):int(L2_off[l]) + NC * kp,
                                  :].opt()],
                )
                last_ag[0] = cc

            # MLP head for this level (fills the AllGather latency)
            for g, w in enumerate(widths):
                emit_mlp(l, g, w, t4g[g])

            # seeds + old scatter matmuls for the next level (also fill)
            if l + 1 < NL:
                emit_seeds(l + 1)
                hg_o, oh_o = Hg_old[l + 1]
                emit_chunks(l + 1, "old", hg_o, oh_o)

        for pl in (rpool, tpool, qpool, ppool, hpool, opool, gpool, spool,
                   cpool):
            pl.release()

    nc.compile()
    return nc


# ---------------------------------------------------------------------------
# Entry point
# ---------------------------------------------------------------------------

def _run(inputs, trace=False):
    from concourse.bass_utils import run_bass_kernel_spmd

    fl = np.asarray(inputs["forward_level"])
    num_levels = int(fl.max()) + 1
    sched = _preprocess(fl, inputs["edge_index"], num_levels)
    wts = _prep_weights(inputs)
    bf16 = np.float16
    iota16 = np.tile(np.arange(1536, dtype=np.float16)[None, :], (128, 1))

    key = (sched["N"], sched["TC"], sched["RC"], sched["sumVc"],
           tuple(int(x) for x in sched["Vc"]),
           tuple((0 if lv is None else
                  (0 if lv["old"] is None else len(lv["old"]["incids"]),
                   0 if lv["fresh"] is None else len(lv["fresh"]["incids"])))
                 for lv in sched["levels"]))
    if key not in _COMPILED:
        _COMPILED[key] = _build(sched)
    nc = _COMPILED[key]

    in_maps = []
    for c in range(NC):
        in_maps.append({
            "wmat": wts["wmat"], "vcols": wts["vcols"],
            "vcolsb": wts["vcolsb"], "rowc": wts["rowc"], "vr3": wts["vr3"],
            "cnts": sched["cnts"][c].astype(bf16),
            "idxs": sched["idxs"][c],
            "rnks": sched["rnks"][c],
            "iota16": iota16,
        })

    res = run_bass_kernel_spmd(nc, in_maps, core_ids=list(range(NC)),
                               trace=trace)

    NL = sched["NL"]
    L_off, Vc, Voff = sched["L_off"], sched["Vc"], sched["Voff"]
    node_of_rank = sched["node_of_rank"]
    out = np.zeros(sched["N"], np.float32)
    out[sched["lvl0"]] = wts["pred0"]
    out[sched["constn"]] = wts["predz"]
    for c in range(NC):
        oc = res.results[c]["pred"]
        for l in range(1, NL):
            gr = int(L_off[l]) + c * int(Vc[l]) + np.arange(int(Vc[l]))
            nd = node_of_rank[gr]
            m = nd >= 0
            out[nd[m]] = oc[int(Voff[l]):int(Voff[l]) + int(Vc[l])][m]
    return out[:, None], res


def kernel(**inputs):
    out, _ = _run(inputs, trace=False)
    return out


# revision 18
# speedup vs baseline: 1.0172x; 1.0172x over previous
"""DeepSAT GNN message-passing kernel for 8 Trainium2 NeuronCores.

Algorithm notes (validated numerically against the reference):
  - Every node is updated exactly once, at step l = forward_level (levels
    1..19; level-0 nodes keep h0 forever). At update time the node's own
    hidden state is still h0, so the GRU "hidden side" gates are constant
    vectors computable on the host.
  - Nodes with deg==0 (no in-edges) and level>=1 get msg=0, hence a single
    constant h_z = GRU(0, h0); their prediction MLP(h_z) is a host-side
    constant, as is MLP(h0) for level-0 nodes. Both node classes are
    excluded from the device rank space entirely; edges sourced at them
    enter the seed counts (n0 for h0-sources, nz for h_z-sources).
  - msg_i = W @ (S_i + n0_i*h0 + nz_i*h_z) + deg_i*b. With u = W^-1 b this
    folds to msg_i = W @ S'_i, S'_i = S_i + n0_i*h0 + nz_i*h_z + deg_i*u,
    so the per-gate input is gi_g = (wih_g @ W) @ S'_i + bih_g.
  - The GRU output is written as h = h0 + t4, t4 = sigm(-z)*(tanh(n) - h0).
    Only t4 is produced on-engine; +h0 is folded into the post-transpose
    copy (row broadcast) and W1@h0 into the MLP's first bias.
  - Everything flows in fp16 (fp32 PSUM accumulation): 4x matmul
    throughput vs fp32, half the AllGather/gather bytes, and fp16's
    11-bit mantissa keeps the end-to-end relative error at ~7e-4.
  - Gathers use the SWDGE dma_gather instruction: ONE instruction per
    (level, phase, 32k-source-segment) regardless of edge count, versus
    ~1us of fixed descriptor-generation overhead per 128-row indirect DMA.

Device schedule per level l (SPMD on 8 cores):
  gather h[src] for this level's "fresh" edges (src level == l-1),
  accumulate via one-hot matmuls into PSUM (seeded with the n0/nz/deg
  terms), fused GRU producing t4, PE-transpose, +h0, DMA to the AllGather
  input, AllGather into the replicated h_store, then (overlapping the
  collective) the MLP head for this level plus the next level's "old"
  edges (src level < l), whose gather only reads rows below this level's
  slab.
"""

import sys

import numpy as np

sys.path.insert(0, "/opt/trn_rl_repo")

P = 128
D = 128
NC = 8
GW = 512          # psum group width (one bank of fp32)
SEG = 32768       # dma_gather int16 index window (rows)

_COMPILED = {}


# ---------------------------------------------------------------------------
# Host-side math helpers
# ---------------------------------------------------------------------------

def _sigmoid(x):
    return 1.0 / (1.0 + np.exp(-x))


def _prep_weights(inp):
    f64 = np.float64
    W = inp["aggr_w"].astype(f64)
    b = inp["aggr_b"].astype(f64)
    h0 = (inp["emd_w"][:, 0] + inp["emd_b"]).astype(f64)
    wih = inp["gru_wih"].astype(f64)
    whh = inp["gru_whh"].astype(f64)
    bih = inp["gru_bih"].astype(f64)
    bhh = inp["gru_bhh"].astype(f64)
    u = np.linalg.solve(W, b)
    assert np.abs(W @ u - b).max() < 1e-5

    ghc = whh @ h0 + bhh
    hr_c, hz_c, hn_c = ghc[:D], ghc[D:2 * D], ghc[2 * D:]
    bih_r, bih_z, bih_n = bih[:D], bih[D:2 * D], bih[2 * D:]
    WgT = [(wih[g * D:(g + 1) * D] @ W).T for g in range(3)]

    # constant hidden state of zero-in-degree nodes (msg = 0)
    r = _sigmoid(bih_r + hr_c)
    z = _sigmoid(bih_z + hz_c)
    n = np.tanh(bih_n + r * hn_c)
    h_z = (1.0 - z) * n + z * h0

    W1 = inp["w1"].astype(f64)   # [256, 128]
    b1 = inp["b1"].astype(f64)
    W2 = inp["w2"].astype(f64)   # [256, 256]
    b2 = inp["b2"].astype(f64)
    w3 = inp["w3"].astype(f64)   # [1, 256]
    b3 = inp["b3"].astype(f64)

    def mlp(h):
        z1 = np.maximum(W1 @ h + b1, 0.0)
        z2 = np.maximum(W2 @ z1 + b2, 0.0)
        return float(w3[0] @ z2 + b3[0])

    pred0 = mlp(h0)
    predz = mlp(h_z)

    b1p = b1 + W1 @ h0          # W1@h0 folded into the first MLP bias

    bf16 = np.float16

    wblocks = [
        WgT[0], WgT[1], WgT[2], np.diag(hn_c),
        W1[0:128, :].T, W1[128:256, :].T,
        W2[0:128, 0:128].T, W2[0:128, 128:256].T,
        W2[128:256, 0:128].T, W2[128:256, 128:256].T,
        np.eye(128),
    ]
    wmat = np.concatenate(wblocks, axis=1).astype(bf16)   # [128, 11*128]

    vcols = np.stack([
        h0,                      # 0: h0 column (tensor_scalar operand)
        bih_r + hr_c,            # 1: sigmoid bias for r
        -(bih_z + hz_c),         # 2: sigmoid bias for z' (scale = -1)
        bih_n,                   # 3: tanh bias for n
        b1p[0:128],              # 4
        b1p[128:256],            # 5
        b2[0:128],               # 6
        b2[128:256],             # 7
        np.full(128, b3[0]),     # 8: b3 (row 0 used)
    ], axis=1).astype(np.float32)                         # [128, 9]

    vcolsb = np.stack([w3[0, 0:128], w3[0, 128:256]],
                      axis=1).astype(bf16)                # [128, 2]

    rowc = np.zeros((1, 1024), np.float32)
    rowc[0, 0:512] = np.arange(512, dtype=np.float32)     # iota for one-hots
    rowc[0, 512:1024] = np.tile(h0.astype(np.float32), 4)  # +h0 after transpose
    rowc = np.repeat(rowc, 128, axis=0)                   # full-partition tile

    vr3 = np.stack([h0, u, h_z], axis=0).astype(bf16)     # [3, 128] seed lhsT

    return {
        "wmat": wmat, "vcols": vcols, "vcolsb": vcolsb, "rowc": rowc,
        "vr3": vr3, "pred0": pred0, "predz": predz, "bf16": bf16,
    }


WM = {name: i for i, name in enumerate(
    ["WgT_r", "WgT_z", "WgT_n", "diag_hn", "W1Ta", "W1Tb",
     "W2_k0m0", "W2_k1m0", "W2_k0m1", "W2_k1m1", "ident"])}
VC = {name: i for i, name in enumerate(
    ["h0", "bias_r", "nbias_z", "bias_n", "b1pa", "b1pb", "b2a", "b2b",
     "b3"])}


# ---------------------------------------------------------------------------
# Host-side preprocessing: rank space + packed edge tables
# ---------------------------------------------------------------------------

def _preprocess(forward_level, edge_index, num_levels):
    fl = np.asarray(forward_level).astype(np.int64)
    ei = np.asarray(edge_index).astype(np.int64)
    src, dst = ei[0], ei[1]
    N = fl.shape[0]
    NL = num_levels

    deg = np.bincount(dst, minlength=N).astype(np.int64)
    lv_s, lv_d = fl[src], fl[dst]
    constn = (deg == 0) & (fl >= 1)      # h == h_z forever
    lvl0 = fl == 0                       # h == h0 forever
    nonc = ~(constn | lvl0)              # device rank space

    act = (lv_s >= 1) & (lv_s < lv_d)
    gact = act & ~constn[src]            # gathered edges
    zact = act & constn[src]             # const-source edges -> nz count
    nz = np.bincount(dst[zact], minlength=N).astype(np.int64)
    ngact = np.bincount(dst[gact], minlength=N).astype(np.int64)
    n0 = deg - nz - ngact

    is_src = np.zeros(N, bool)
    is_src[src[gact]] = True

    # --- rank space: nonconst nodes sorted by level, levels 1..NL-1 ---
    n_l = np.bincount(fl[nonc], minlength=NL).astype(np.int64)
    n_l[0] = 0
    # +2*NC slack so the per-core interleaved source/non-source split fits
    pad_l = ((n_l + 2 * NC + NC * P - 1) // (NC * P)) * (NC * P)
    pad_l = np.maximum(pad_l, NC * P)
    pad_l[0] = 0
    L_off = np.zeros(NL + 1, np.int64)
    L_off[1:] = np.cumsum(pad_l)
    Vc = (pad_l // NC).astype(np.int64)
    Voff = np.zeros(NL + 1, np.int64)
    Voff[1:] = np.cumsum(Vc)
    nblk = (Vc // P).astype(np.int64)
    sumVc = int(Voff[NL])
    NpadTot = int(L_off[NL])

    # per (level, core): gather-source nodes first, so the AllGather only
    # needs to move the first Kpad rows of each core's shard
    rank = np.full(N, -1, np.int64)
    rank2 = np.full(N, -1, np.int64)     # rank in the compacted h_store
    Kpad = np.zeros(NL, np.int64)
    for l in range(1, NL):
        lv_nodes = np.where(nonc & (fl == l))[0]
        s_nodes = lv_nodes[is_src[lv_nodes]]
        r_nodes = lv_nodes[~is_src[lv_nodes]]
        S, R = len(s_nodes), len(r_nodes)
        s_core = np.arange(S, dtype=np.int64) % NC
        s_pos = np.arange(S, dtype=np.int64) // NC
        Kc = np.bincount(s_core, minlength=NC).astype(np.int64)
        r_core = np.arange(R, dtype=np.int64) % NC
        r_pos = Kc[r_core] + np.arange(R, dtype=np.int64) // NC
        assert R == 0 or r_pos.max() < int(Vc[l])
        rank[s_nodes] = L_off[l] + s_core * Vc[l] + s_pos
        rank[r_nodes] = L_off[l] + r_core * Vc[l] + r_pos
        if S > 0:
            Kpad[l] = ((int(Kc.max()) + P - 1) // P) * P
            rank2[s_nodes] = s_core * Kpad[l] + s_pos  # + L2_off[l] below
    L2_off = np.zeros(NL + 1, np.int64)
    L2_off[1:] = np.cumsum(NC * Kpad)
    NpadTot2 = int(L2_off[NL])
    sn = np.where(rank2 >= 0)[0]
    rank2[sn] += L2_off[fl[sn]]
    node_of_rank = np.full(NpadTot, -1, np.int64)
    nodes = np.where(nonc)[0]
    node_of_rank[rank[nodes]] = nodes

    # --- per-core seed counts [NC, 3, sumVc] (rows: n0, deg, nz) ---
    cnts = np.zeros((NC, 3, sumVc), np.float32)
    for c in range(NC):
        grs = []
        for l in range(NL):
            grs.append(L_off[l] + c * Vc[l] + np.arange(Vc[l]))
        gr = np.concatenate(grs)
        nd = node_of_rank[gr]
        m = nd >= 0
        cnts[c, 0, m] = n0[nd[m]]
        cnts[c, 1, m] = deg[nd[m]]
        cnts[c, 2, m] = nz[nd[m]]

    # --- packed edge tables ---
    er = np.where(gact)[0]
    e_lvl = lv_d[er]
    e_srcrank = rank2[src[er]]
    assert e_srcrank.min(initial=0) >= 0
    e_dstrank = rank[dst[er]]
    e_local = e_dstrank - L_off[e_lvl]
    e_core = e_local // Vc[e_lvl]
    e_wl = e_local % Vc[e_lvl]
    e_fresh = lv_s[er] == (e_lvl - 1)

    idx_cols = [[] for _ in range(NC)]   # int16 [128, k] blocks
    rnk_cols = [[] for _ in range(NC)]   # f32 [128] columns
    icol = [0]
    rcol = [0]

    def pack_gather(base, sel_core_edges, nch):
        """Append idx columns for one dma_gather; returns icol0."""
        i0 = icol[0]
        for c in range(NC):
            es = sel_core_edges[c]
            vals = np.zeros(nch * P, np.int64)
            vals[:len(es)] = e_srcrank[es] - base
            assert vals.max(initial=0) < SEG and vals.min(initial=0) >= 0
            blk = np.zeros((128, nch * 8), np.int16)
            # HW (queue 0) reads index i at partition 16 + i%16, col i//16;
            # the interpreter reads partitions 0..15 — fill both.
            blk[:16, :] = vals.astype(np.int16).reshape(nch * 8, 16).T
            blk[16:32, :] = blk[:16, :]
            idx_cols[c].append(blk)
        icol[0] += nch * 8
        return i0

    def pack_phase(l, sel, is_fresh):
        """Pack one (level, phase); returns phase dict or None."""
        percore_all = [er_idx[sel & (e_core == c)] for c in range(NC)]
        if all(len(x) == 0 for x in percore_all):
            return None
        if is_fresh:
            # fresh: single segment based at the previous level's compacted
            # slab, which spans [L2_off[l-1], L2_off[l])
            bases = [int(L2_off[l - 1])]
            bound = int(L2_off[l])
        else:
            segs = sorted(set((e_srcrank[sel] // SEG).tolist()))
            bases = [int(s * SEG) for s in segs]
            bound = int(L2_off[l - 1])
        gathers = []
        incids = []
        ch0 = 0
        for base in bases:
            rows = min(SEG, bound - base)
            sel_b = sel & (e_srcrank >= base) & (e_srcrank < base + SEG)
            percore = []
            for c in range(NC):
                es = er_idx[sel_b & (e_core == c)]
                es = es[np.argsort(e_wl[es], kind="stable")]
                percore.append(es)
            cnt = max(len(x) for x in percore)
            if cnt == 0:
                continue
            nch = (cnt + P - 1) // P
            i0 = pack_gather(base, percore, nch)
            for j in range(nch):
                groups = set()
                chunk_es = [x[j * P:(j + 1) * P] for x in percore]
                for es in chunk_es:
                    groups.update((e_wl[es] // GW).tolist())
                for g in sorted(groups):
                    for c in range(NC):
                        es = chunk_es[c]
                        rv = np.full(P, -1.0, np.float16)
                        ing = (e_wl[es] // GW) == g
                        rv[np.where(ing)[0]] = (e_wl[es[ing]] - g * GW)
                        rnk_cols[c].append(rv)
                    incids.append([ch0 + j, int(g), rcol[0], False])
                    rcol[0] += 1
            gathers.append((base, rows, nch, i0, ch0))
            ch0 += nch
        if not gathers:
            return None
        return {"gathers": gathers, "incids": incids, "nch_total": ch0}

    er_idx = np.arange(len(er), dtype=np.int64)
    levels = [None] * NL
    for l in range(1, NL):
        in_lvl = e_lvl == l
        old = pack_phase(l, in_lvl & ~e_fresh, False)
        fresh = pack_phase(l, in_lvl & e_fresh, True)
        ngrp = (int(Vc[l]) + GW - 1) // GW
        # stop-flag per psum group: seeds -> old -> fresh (emission order)
        seed_stop = [True] * ngrp
        for ph in (old, fresh):
            if ph is None:
                continue
            for it in ph["incids"]:
                seed_stop[it[1]] = False
        lastmark = {}
        for name, ph in (("old", old), ("fresh", fresh)):
            if ph is None:
                continue
            for i, it in enumerate(ph["incids"]):
                lastmark[it[1]] = (name, i)
        for name, ph in (("old", old), ("fresh", fresh)):
            if ph is None:
                continue
            for i, it in enumerate(ph["incids"]):
                it[3] = lastmark.get(it[1]) == (name, i)
        levels[l] = {"old": old, "fresh": fresh, "seed_stop": seed_stop,
                     "ngrp": ngrp}

    TC = max(icol[0], 8)
    RC = max(rcol[0], 1)
    idxs = np.zeros((NC, 128, TC), np.int16)
    rnks = np.full((NC, 128, RC), -1.0, np.float16)
    for c in range(NC):
        if idx_cols[c]:
            idxs[c, :, :icol[0]] = np.concatenate(idx_cols[c], axis=1)
        if rnk_cols[c]:
            rnks[c, :, :rcol[0]] = np.stack(rnk_cols[c], axis=1)

    return {
        "N": N, "NL": NL, "n_l": n_l, "pad": pad_l, "L_off": L_off,
        "Vc": Vc, "Voff": Voff, "nblk": nblk, "sumVc": sumVc,
        "NpadTot": NpadTot, "TC": TC, "RC": RC,
        "Kpad": Kpad, "L2_off": L2_off, "NpadTot2": NpadTot2,
        "rank2": rank2,
        "levels": levels, "idxs": idxs, "rnks": rnks, "cnts": cnts,
        "node_of_rank": node_of_rank, "lvl0": lvl0, "constn": constn,
    }


# ---------------------------------------------------------------------------
# Bass program
# ---------------------------------------------------------------------------

def _build(sched):
    import concourse.bacc as bacc
    import concourse.tile as tile
    from concourse import bass, mybir

    f32 = mybir.dt.float32
    bf16 = mybir.dt.float16
    i16 = mybir.dt.int16
    AF = mybir.ActivationFunctionType
    OP = mybir.AluOpType
    NL = sched["NL"]
    L_off = sched["L_off"]
    Vc = sched["Vc"]
    Voff = sched["Voff"]
    pad = sched["pad"]
    TC = sched["TC"]
    RC = sched["RC"]
    sumVc = sched["sumVc"]
    NpadTot = sched["NpadTot"]
    Kpad = sched["Kpad"]
    L2_off = sched["L2_off"]
    NpadTot2 = sched["NpadTot2"]
    Vcmax = int(Vc.max())
    Kpadmax = max(int(Kpad.max()), P)
    RG = [list(range(NC))]

    nc = bacc.Bacc("TRN2", target_bir_lowering=False, debug=False,
                   enable_asserts=False, num_devices=NC)

    wmat_d = nc.dram_tensor("wmat", [P, P * len(WM)], bf16, kind="ExternalInput")
    vc_d = nc.dram_tensor("vcols", [P, len(VC)], f32, kind="ExternalInput")
    vcb_d = nc.dram_tensor("vcolsb", [P, 2], bf16, kind="ExternalInput")
    rowc_d = nc.dram_tensor("rowc", [P, 1024], f32, kind="ExternalInput")
    vr3_d = nc.dram_tensor("vr3", [3, P], bf16, kind="ExternalInput")
    cnt_d = nc.dram_tensor("cnts", [3, sumVc], bf16, kind="ExternalInput")
    idx_d = nc.dram_tensor("idxs", [P, TC], i16, kind="ExternalInput")
    rnk_d = nc.dram_tensor("rnks", [P, RC], bf16, kind="ExternalInput")
    io16_d = nc.dram_tensor("iota16", [P, 1536], bf16, kind="ExternalInput")
    pred_d = nc.dram_tensor("pred", [sumVc], f32, kind="ExternalOutput")
    h_store = nc.dram_tensor("h_store", [NpadTot2, D], bf16, kind="Internal",
                             addr_space="Shared")
    ag_in = [nc.dram_tensor(f"ag_in{i}", [Kpadmax, D], bf16, kind="Internal")
             for i in range(2)]

    with tile.TileContext(nc) as tc:
        cpool = tc.alloc_tile_pool(name="const", bufs=1)
        spool = tc.alloc_tile_pool(name="sbuf", bufs=2)
        gpool = tc.alloc_tile_pool(name="gath", bufs=2)
        opool = tc.alloc_tile_pool(name="oh", bufs=2)
        hpool = tc.alloc_tile_pool(name="hnew", bufs=6)
        ppool = tc.alloc_tile_pool(name="psS", bufs=3, space="PSUM")
        qpool = tc.alloc_tile_pool(name="psG", bufs=3, space="PSUM")
        tpool = tc.alloc_tile_pool(name="psT", bufs=1, space="PSUM")
        rpool = tc.alloc_tile_pool(name="psP", bufs=1, space="PSUM")

        # ---- load constants ----
        wm = cpool.tile([P, P * len(WM)], bf16, tag="wm")
        nc.sync.dma_start(out=wm[:], in_=wmat_d[:])
        vc = cpool.tile([P, len(VC)], f32, tag="vc")
        nc.sync.dma_start(out=vc[:], in_=vc_d[:])
        vcb = cpool.tile([P, 2], bf16, tag="vcb")
        nc.sync.dma_start(out=vcb[:], in_=vcb_d[:])
        rowc = cpool.tile([P, 1024], f32, tag="rowc")
        nc.sync.dma_start(out=rowc[:], in_=rowc_d[:])
        vr3 = cpool.tile([3, P], bf16, tag="vr3")
        nc.sync.dma_start(out=vr3[:], in_=vr3_d[:])
        idxs = cpool.tile([P, TC], i16, tag="idxs")
        nc.sync.dma_start(out=idxs[:], in_=idx_d[:])
        rnks = cpool.tile([P, RC], bf16, tag="rnks")
        nc.sync.dma_start(out=rnks[:], in_=rnk_d[:])
        io16 = cpool.tile([P, 1536], bf16, tag="io16")
        nc.sync.dma_start(out=io16[:], in_=io16_d[:])

        def wmb(name):
            return wm[:, WM[name] * P:(WM[name] + 1) * P]

        def vcc(name):
            return vc[:, VC[name]:VC[name] + 1]

        # ---- per-level state ----
        S_ps = [None] * NL
        Hg_old = [None] * (NL + 1)
        last_ag = [None]

        def grp_widths(l):
            ws = []
            v = int(Vc[l])
            while v > 0:
                ws.append(min(GW, v))
                v -= GW
            return ws

        def emit_gathers(info, which, bound_level):
            """One dma_gather per source segment; reads h_store below
            L_off[bound_level]."""
            ph = info[which]
            if ph is None:
                return None
            hg = gpool.tile([P, ph["nch_total"] * D], bf16, tag="hg_" + which)
            for (base, rows, nch, i0, ch0) in ph["gathers"]:
                gi = nc.gpsimd.dma_gather(
                    out_ap=hg[:, ch0 * D:(ch0 + nch) * D].rearrange(
                        "p (k d) -> p k d", d=D),
                    in_ap=h_store[base:base + rows, :],
                    idxs_ap=idxs[:, i0:i0 + nch * 8],
                    num_idxs=nch * P,
                    num_idxs_reg=nch * P,
                    elem_size=D,
                )
                # dynamic DRAM reads are not region-tracked by Tile's shadow
                # memory: pin the RAW edge vs the latest AllGather by hand
                if last_ag[0] is not None:
                    tile.add_dep_helper(gi.ins, last_ag[0].ins, sync=True,
                                        reason="gather reads AllGather output")
            return hg

        def emit_onehots(info, which, widths):
            ph = info[which]
            if ph is None:
                return None
            n = len(ph["incids"])
            oh = opool.tile([P, n * GW], bf16, tag="oh_" + which)
            for i, (ch, g, rc, stop) in enumerate(ph["incids"]):
                w = widths[g]
                nc.vector.tensor_tensor(
                    out=oh[:, i * GW:i * GW + w],
                    in0=rnks[:, rc:rc + 1].to_broadcast([P, w]),
                    in1=io16[:, 0:w],
                    op=OP.is_equal,
                )
            return oh

        def emit_chunks(l, which, hg, oh):
            info = sched["levels"][l]
            ph = info[which]
            if ph is None:
                return
            widths = grp_widths(l)
            for i, (ch, g, rc, stop) in enumerate(ph["incids"]):
                w = widths[g]
                nc.tensor.matmul(
                    out=S_ps[l][g][:, :w],
                    lhsT=hg[:, ch * D:(ch + 1) * D],
                    rhs=oh[:, i * GW:i * GW + w],
                    start=False, stop=stop, skip_group_check=True)

        def emit_seeds(l):
            info = sched["levels"][l]
            tiles = []
            v = int(Vc[l])
            off = int(Voff[l])
            cm = spool.tile([3, Vcmax], bf16, tag="cm")
            nc.sync.dma_start(out=cm[0:3, :v], in_=cnt_d[0:3, off:off + v])
            for g, w in enumerate(grp_widths(l)):
                sp = ppool.tile([P, GW], f32, tag="S", space="PSUM")
                nc.tensor.matmul(
                    out=sp[:, :w], lhsT=vr3[0:3, :],
                    rhs=cm[0:3, g * GW:g * GW + w],
                    start=True, stop=info["seed_stop"][g],
                    skip_group_check=True)
                tiles.append(sp)
            S_ps[l] = tiles

        def emit_mlp(l, g, w, rhs_sb):
            """MLP head for one 512-group. rhs_sb is t4 (h - h0); the W1@h0
            part lives in the b1p bias."""
            z1s = []
            for half in ("a", "b"):
                zp = qpool.tile([P, GW], f32, tag="G", space="PSUM")
                nc.tensor.matmul(out=zp[:, :w], lhsT=wmb("W1T" + half),
                                 rhs=rhs_sb[:, :w], start=True, stop=True)
                zs = spool.tile([P, GW], bf16, tag="z1" + half)
                nc.scalar.activation(out=zs[:, :w], in_=zp[:, :w],
                                     func=AF.Relu, bias=vcc("b1p" + half))
                z1s.append(zs)
            z2s = []
            for mi, mh in enumerate(("m0", "m1")):
                zp = qpool.tile([P, GW], f32, tag="G", space="PSUM")
                nc.tensor.matmul(out=zp[:, :w], lhsT=wmb("W2_k0" + mh),
                                 rhs=z1s[0][:, :w], start=True, stop=False)
                nc.tensor.matmul(out=zp[:, :w], lhsT=wmb("W2_k1" + mh),
                                 rhs=z1s[1][:, :w], start=False, stop=True)
                zs = spool.tile([P, GW], bf16, tag="z2" + mh)
                nc.scalar.activation(out=zs[:, :w], in_=zp[:, :w], func=AF.Relu,
                                     bias=vcc("b2a" if mi == 0 else "b2b"))
                z2s.append(zs)
            pp = rpool.tile([1, GW], f32, tag="pred", space="PSUM")
            nc.tensor.matmul(out=pp[:, :w], lhsT=vcb[:, 0:1], rhs=z2s[0][:, :w],
                             start=True, stop=False)
            nc.tensor.matmul(out=pp[:, :w], lhsT=vcb[:, 1:2], rhs=z2s[1][:, :w],
                             start=False, stop=True)
            ps = spool.tile([1, GW], f32, tag="psb")
            nc.scalar.activation(out=ps[:, :w], in_=pp[:, :w], func=AF.Identity,
                                 bias=vc[0:1, VC["b3"]:VC["b3"] + 1])
            off = int(Voff[l]) + g * GW
            nc.sync.dma_start(out=pred_d[off:off + w], in_=ps[0:1, :w])

        # seeds for level 1 (no old/fresh edges possible at level 1)
        emit_seeds(1)

        # ================= levels 1..NL-1 =================
        for l in range(1, NL):
            info = sched["levels"][l]
            widths = grp_widths(l)

            # fresh gather + one-hots + scatter matmuls for this level
            hg_f = emit_gathers(info, "fresh", l)
            oh_f = emit_onehots(info, "fresh", grp_widths(l))
            emit_chunks(l, "fresh", hg_f, oh_f)

            # old gather for the NEXT level: sources are below L_off[l], so
            # the dma_gather runs before this level's AllGather on the queue
            if l + 1 < NL:
                ninfo = sched["levels"][l + 1]
                Hg_old[l + 1] = (
                    emit_gathers(ninfo, "old", l),
                    emit_onehots(ninfo, "old", grp_widths(l + 1)),
                )

            # GRU per group: t4 = sigm(-z) * (tanh(n) - h0);  h = h0 + t4
            t4g = []
            for g, w in enumerate(widths):
                ssb = spool.tile([P, GW], bf16, tag="Ssb")
                nc.vector.tensor_copy(out=ssb[:, :w], in_=S_ps[l][g][:, :w])

                gr = qpool.tile([P, GW], f32, tag="G", space="PSUM")
                nc.tensor.matmul(out=gr[:, :w], lhsT=wmb("WgT_r"),
                                 rhs=ssb[:, :w], start=True, stop=True)
                gz = qpool.tile([P, GW], f32, tag="G", space="PSUM")
                nc.tensor.matmul(out=gz[:, :w], lhsT=wmb("WgT_z"),
                                 rhs=ssb[:, :w], start=True, stop=True)
                gn = qpool.tile([P, GW], f32, tag="G", space="PSUM")
                nc.tensor.matmul(out=gn[:, :w], lhsT=wmb("WgT_n"),
                                 rhs=ssb[:, :w], start=True, stop=False)

                rsb = spool.tile([P, GW], bf16, tag="rsb")
                nc.scalar.activation(out=rsb[:, :w], in_=gr[:, :w],
                                     func=AF.Sigmoid, bias=vcc("bias_r"))
                zsb = spool.tile([P, GW], f32, tag="zsb")
                nc.scalar.activation(out=zsb[:, :w], in_=gz[:, :w],
                                     func=AF.Sigmoid, bias=vcc("nbias_z"),
                                     scale=-1.0)
                nc.tensor.matmul(out=gn[:, :w], lhsT=wmb("diag_hn"),
                                 rhs=rsb[:, :w], start=False, stop=True)
                nsb = spool.tile([P, GW], f32, tag="nsb")
                nc.scalar.activation(out=nsb[:, :w], in_=gn[:, :w],
                                     func=AF.Tanh, bias=vcc("bias_n"))

                t3 = spool.tile([P, GW], f32, tag="t3")
                nc.vector.tensor_scalar(out=t3[:, :w], in0=nsb[:, :w],
                                        scalar1=vcc("h0"), scalar2=None,
                                        op0=OP.subtract)
                t4 = hpool.tile([P, GW], bf16, tag="t4")
                nc.vector.tensor_tensor(out=t4[:, :w], in0=t3[:, :w],
                                        in1=zsb[:, :w], op=OP.mult)
                t4g.append(t4)

            # transpose+stage only the source prefix (first Kpad rows of
            # this core's shard), AllGather it into the compacted h_store
            # (skipped for the last level: nothing reads it)
            kp = int(Kpad[l])
            if l < NL - 1 and kp > 0:
                agt = ag_in[l % 2]
                for g, w in enumerate(widths):
                    ws = min(w, kp - g * GW)
                    if ws <= 0:
                        break
                    tp = tpool.tile([P, GW], bf16, tag="tp", space="PSUM")
                    nb = ws // P
                    for b in range(nb):
                        nc.tensor.transpose(
                            out=tp[:, b * P:(b + 1) * P],
                            in_=t4g[g][:, b * P:(b + 1) * P],
                            identity=wmb("ident"))
                    tps = spool.tile([P, GW], bf16, tag="tps")
                    nc.vector.tensor_tensor(
                        out=tps[:, :ws], in0=tp[:, :ws],
                        in1=rowc[:, 512:512 + ws],
                        op=OP.add)
                    for b in range(nb):
                        row = g * GW + b * P
                        nc.sync.dma_start(out=agt[row:row + P, :],
                                          in_=tps[:, b * P:(b + 1) * P])
                cc = nc.gpsimd.collective_compute(
                    "AllGather", bass.mybir.AluOpType.bypass,
                    replica_groups=RG,
                    ins=[agt[0:kp, :].opt()],
                    outs=[h_store[int(L2_off[l]):int(L2_off[l]) + NC * kp,
                                  :].opt()],
                )
                last_ag[0] = cc

            # MLP head for this level (fills the AllGather latency)
            for g, w in enumerate(widths):
                emit_mlp(l, g, w, t4g[g])

            # seeds + old scatter matmuls for the next level (also fill)
            if l + 1 < NL:
                emit_seeds(l + 1)
                hg_o, oh_o = Hg_old[l + 1]
                emit_chunks(l + 1, "old", hg_o, oh_o)

        for pl in (rpool, tpool, qpool, ppool, hpool, opool, gpool, spool,
                   cpool):
            pl.release()

    nc.compile()
    return nc


# ---------------------------------------------------------------------------
# Entry point
# ---------------------------------------------------------------------------

def _run(inputs, trace=False):
    from concourse.bass_utils import run_bass_kernel_spmd

    fl = np.asarray(inputs["forward_level"])
    num_levels = int(fl.max()) + 1
    sched = _preprocess(fl, inputs["edge_index"], num_levels)
    wts = _prep_weights(inputs)
    bf16 = np.float16
    iota16 = np.tile(np.arange(1536, dtype=np.float16)[None, :], (128, 1))

    key = (sched["N"], sched["TC"], sched["RC"], sched["sumVc"],
           tuple(int(x) for x in sched["Vc"]),
           tuple((0 if lv is None else
                  (0 if lv["old"] is None else len(lv["old"]["incids"]),
                   0 if lv["fresh"] is None else len(lv["fresh"]["incids"])))
                 for lv in sched["levels"]))
    if key not in _COMPILED:
        _COMPILED[key] = _build(sched)
    nc = _COMPILED[key]

    in_maps = []
    for c in range(NC):
        in_maps.append({
            "wmat": wts["wmat"], "vcols": wts["vcols"],
            "vcolsb": wts["vcolsb"], "rowc": wts["rowc"], "vr3": wts["vr3"],
            "cnts": sched["cnts"][c].astype(bf16),
            "idxs": sched["idxs"][c],
            "rnks": sched["rnks"][c],
            "iota16": iota16,
        })

    res = run_bass_kernel_spmd(nc, in_maps, core_ids=list(range(NC)),
                               trace=trace)

    NL = sched["NL"]
    L_off, Vc, Voff = sched["L_off"], sched["Vc"], sched["Voff"]
    node_of_rank = sched["node_of_rank"]
    out = np.zeros(sched["N"], np.float32)
    out[sched["lvl0"]] = wts["pred0"]
    out[sched["constn"]] = wts["predz"]
    for c in range(NC):
        oc = res.results[c]["pred"]
        for l in range(1, NL):
            gr = int(L_off[l]) + c * int(Vc[l]) + np.arange(int(Vc[l]))
            nd = node_of_rank[gr]
            m = nd >= 0
            out[nd[m]] = oc[int(Voff[l]):int(Voff[l]) + int(Vc[l])][m]
    return out[:, None], res


def kernel(**inputs):
    out, _ = _run(inputs, trace=False)
    return out


# revision 19
# speedup vs baseline: 1.0175x; 1.0003x over previous
"""DeepSAT GNN message-passing kernel for 8 Trainium2 NeuronCores.

Algorithm notes (validated numerically against the reference):
  - Every node is updated exactly once, at step l = forward_level (levels
    1..19; level-0 nodes keep h0 forever). At update time the node's own
    hidden state is still h0, so the GRU "hidden side" gates are constant
    vectors computable on the host.
  - Nodes with deg==0 (no in-edges) and level>=1 get msg=0, hence a single
    constant h_z = GRU(0, h0); their prediction MLP(h_z) is a host-side
    constant, as is MLP(h0) for level-0 nodes. Both node classes are
    excluded from the device rank space entirely; edges sourced at them
    enter the seed counts (n0 for h0-sources, nz for h_z-sources).
  - msg_i = W @ (S_i + n0_i*h0 + nz_i*h_z) + deg_i*b. With u = W^-1 b this
    folds to msg_i = W @ S'_i, S'_i = S_i + n0_i*h0 + nz_i*h_z + deg_i*u,
    so the per-gate input is gi_g = (wih_g @ W) @ S'_i + bih_g.
  - The GRU output is written as h = h0 + t4, t4 = sigm(-z)*(tanh(n) - h0).
    Only t4 is produced on-engine; +h0 is folded into the post-transpose
    copy (row broadcast) and W1@h0 into the MLP's first bias.
  - Everything flows in fp16 (fp32 PSUM accumulation): 4x matmul
    throughput vs fp32, half the AllGather/gather bytes, and fp16's
    11-bit mantissa keeps the end-to-end relative error at ~7e-4.
  - Gathers use the SWDGE dma_gather instruction: ONE instruction per
    (level, phase, 32k-source-segment) regardless of edge count, versus
    ~1us of fixed descriptor-generation overhead per 128-row indirect DMA.

Device schedule per level l (SPMD on 8 cores):
  gather h[src] for this level's "fresh" edges (src level == l-1),
  accumulate via one-hot matmuls into PSUM (seeded with the n0/nz/deg
  terms), fused GRU producing t4, PE-transpose, +h0, DMA to the AllGather
  input, AllGather into the replicated h_store, then (overlapping the
  collective) the MLP head for this level plus the next level's "old"
  edges (src level < l), whose gather only reads rows below this level's
  slab.
"""

import sys

import numpy as np

sys.path.insert(0, "/opt/trn_rl_repo")

P = 128
D = 128
NC = 8
GW = 512          # psum group width (one bank of fp32)
SEG = 32768       # dma_gather int16 index window (rows)

_COMPILED = {}


# ---------------------------------------------------------------------------
# Host-side math helpers
# ---------------------------------------------------------------------------

def _sigmoid(x):
    return 1.0 / (1.0 + np.exp(-x))


def _prep_weights(inp):
    f64 = np.float64
    W = inp["aggr_w"].astype(f64)
    b = inp["aggr_b"].astype(f64)
    h0 = (inp["emd_w"][:, 0] + inp["emd_b"]).astype(f64)
    wih = inp["gru_wih"].astype(f64)
    whh = inp["gru_whh"].astype(f64)
    bih = inp["gru_bih"].astype(f64)
    bhh = inp["gru_bhh"].astype(f64)
    u = np.linalg.solve(W, b)
    assert np.abs(W @ u - b).max() < 1e-5

    ghc = whh @ h0 + bhh
    hr_c, hz_c, hn_c = ghc[:D], ghc[D:2 * D], ghc[2 * D:]
    bih_r, bih_z, bih_n = bih[:D], bih[D:2 * D], bih[2 * D:]
    WgT = [(wih[g * D:(g + 1) * D] @ W).T for g in range(3)]

    # constant hidden state of zero-in-degree nodes (msg = 0)
    r = _sigmoid(bih_r + hr_c)
    z = _sigmoid(bih_z + hz_c)
    n = np.tanh(bih_n + r * hn_c)
    h_z = (1.0 - z) * n + z * h0

    W1 = inp["w1"].astype(f64)   # [256, 128]
    b1 = inp["b1"].astype(f64)
    W2 = inp["w2"].astype(f64)   # [256, 256]
    b2 = inp["b2"].astype(f64)
    w3 = inp["w3"].astype(f64)   # [1, 256]
    b3 = inp["b3"].astype(f64)

    def mlp(h):
        z1 = np.maximum(W1 @ h + b1, 0.0)
        z2 = np.maximum(W2 @ z1 + b2, 0.0)
        return float(w3[0] @ z2 + b3[0])

    pred0 = mlp(h0)
    predz = mlp(h_z)

    b1p = b1 + W1 @ h0          # W1@h0 folded into the first MLP bias

    bf16 = np.float16

    wblocks = [
        WgT[0], WgT[1], WgT[2], np.diag(hn_c),
        W1[0:128, :].T, W1[128:256, :].T,
        W2[0:128, 0:128].T, W2[0:128, 128:256].T,
        W2[128:256, 0:128].T, W2[128:256, 128:256].T,
        np.eye(128),
    ]
    wmat = np.concatenate(wblocks, axis=1).astype(bf16)   # [128, 11*128]

    vcols = np.stack([
        h0,                      # 0: h0 column (tensor_scalar operand)
        bih_r + hr_c,            # 1: sigmoid bias for r
        -(bih_z + hz_c),         # 2: sigmoid bias for z' (scale = -1)
        bih_n,                   # 3: tanh bias for n
        b1p[0:128],              # 4
        b1p[128:256],            # 5
        b2[0:128],               # 6
        b2[128:256],             # 7
        np.full(128, b3[0]),     # 8: b3 (row 0 used)
    ], axis=1).astype(np.float32)                         # [128, 9]

    vcolsb = np.stack([w3[0, 0:128], w3[0, 128:256]],
                      axis=1).astype(bf16)                # [128, 2]

    rowc = np.zeros((1, 1024), np.float32)
    rowc[0, 0:512] = np.arange(512, dtype=np.float32)     # iota for one-hots
    rowc[0, 512:1024] = np.tile(h0.astype(np.float32), 4)  # +h0 after transpose
    rowc = np.repeat(rowc, 128, axis=0)                   # full-partition tile

    vr3 = np.stack([h0, u, h_z], axis=0).astype(bf16)     # [3, 128] seed lhsT

    return {
        "wmat": wmat, "vcols": vcols, "vcolsb": vcolsb, "rowc": rowc,
        "vr3": vr3, "pred0": pred0, "predz": predz, "bf16": bf16,
    }


WM = {name: i for i, name in enumerate(
    ["WgT_r", "WgT_z", "WgT_n", "diag_hn", "W1Ta", "W1Tb",
     "W2_k0m0", "W2_k1m0", "W2_k0m1", "W2_k1m1", "ident"])}
VC = {name: i for i, name in enumerate(
    ["h0", "bias_r", "nbias_z", "bias_n", "b1pa", "b1pb", "b2a", "b2b",
     "b3"])}


# ---------------------------------------------------------------------------
# Host-side preprocessing: rank space + packed edge tables
# ---------------------------------------------------------------------------

def _preprocess(forward_level, edge_index, num_levels):
    fl = np.asarray(forward_level).astype(np.int64)
    ei = np.asarray(edge_index).astype(np.int64)
    src, dst = ei[0], ei[1]
    N = fl.shape[0]
    NL = num_levels

    deg = np.bincount(dst, minlength=N).astype(np.int64)
    lv_s, lv_d = fl[src], fl[dst]
    constn = (deg == 0) & (fl >= 1)      # h == h_z forever
    lvl0 = fl == 0                       # h == h0 forever
    nonc = ~(constn | lvl0)              # device rank space

    act = (lv_s >= 1) & (lv_s < lv_d)
    gact = act & ~constn[src]            # gathered edges
    zact = act & constn[src]             # const-source edges -> nz count
    nz = np.bincount(dst[zact], minlength=N).astype(np.int64)
    ngact = np.bincount(dst[gact], minlength=N).astype(np.int64)
    n0 = deg - nz - ngact

    is_src = np.zeros(N, bool)
    is_src[src[gact]] = True

    # --- rank space: nonconst nodes sorted by level, levels 1..NL-1 ---
    n_l = np.bincount(fl[nonc], minlength=NL).astype(np.int64)
    n_l[0] = 0
    # +2*NC slack so the per-core interleaved source/non-source split fits
    pad_l = ((n_l + 2 * NC + NC * P - 1) // (NC * P)) * (NC * P)
    pad_l = np.maximum(pad_l, NC * P)
    pad_l[0] = 0
    L_off = np.zeros(NL + 1, np.int64)
    L_off[1:] = np.cumsum(pad_l)
    Vc = (pad_l // NC).astype(np.int64)
    Voff = np.zeros(NL + 1, np.int64)
    Voff[1:] = np.cumsum(Vc)
    nblk = (Vc // P).astype(np.int64)
    sumVc = int(Voff[NL])
    NpadTot = int(L_off[NL])

    # per (level, core): gather-source nodes first, so the AllGather only
    # needs to move the first Kpad rows of each core's shard
    rank = np.full(N, -1, np.int64)
    rank2 = np.full(N, -1, np.int64)     # rank in the compacted h_store
    Kpad = np.zeros(NL, np.int64)
    for l in range(1, NL):
        lv_nodes = np.where(nonc & (fl == l))[0]
        s_nodes = lv_nodes[is_src[lv_nodes]]
        r_nodes = lv_nodes[~is_src[lv_nodes]]
        S, R = len(s_nodes), len(r_nodes)
        s_core = np.arange(S, dtype=np.int64) % NC
        s_pos = np.arange(S, dtype=np.int64) // NC
        Kc = np.bincount(s_core, minlength=NC).astype(np.int64)
        r_core = np.arange(R, dtype=np.int64) % NC
        r_pos = Kc[r_core] + np.arange(R, dtype=np.int64) // NC
        assert R == 0 or r_pos.max() < int(Vc[l])
        rank[s_nodes] = L_off[l] + s_core * Vc[l] + s_pos
        rank[r_nodes] = L_off[l] + r_core * Vc[l] + r_pos
        if S > 0:
            Kpad[l] = ((int(Kc.max()) + P - 1) // P) * P
            rank2[s_nodes] = s_core * Kpad[l] + s_pos  # + L2_off[l] below
    L2_off = np.zeros(NL + 1, np.int64)
    L2_off[1:] = np.cumsum(NC * Kpad)
    NpadTot2 = int(L2_off[NL])
    sn = np.where(rank2 >= 0)[0]
    rank2[sn] += L2_off[fl[sn]]
    node_of_rank = np.full(NpadTot, -1, np.int64)
    nodes = np.where(nonc)[0]
    node_of_rank[rank[nodes]] = nodes

    # --- per-core seed counts [NC, 3, sumVc] (rows: n0, deg, nz) ---
    cnts = np.zeros((NC, 3, sumVc), np.float32)
    for c in range(NC):
        grs = []
        for l in range(NL):
            grs.append(L_off[l] + c * Vc[l] + np.arange(Vc[l]))
        gr = np.concatenate(grs)
        nd = node_of_rank[gr]
        m = nd >= 0
        cnts[c, 0, m] = n0[nd[m]]
        cnts[c, 1, m] = deg[nd[m]]
        cnts[c, 2, m] = nz[nd[m]]

    # --- packed edge tables ---
    er = np.where(gact)[0]
    e_lvl = lv_d[er]
    e_srcrank = rank2[src[er]]
    assert e_srcrank.min(initial=0) >= 0
    e_dstrank = rank[dst[er]]
    e_local = e_dstrank - L_off[e_lvl]
    e_core = e_local // Vc[e_lvl]
    e_wl = e_local % Vc[e_lvl]
    e_fresh = lv_s[er] == (e_lvl - 1)

    idx_cols = [[] for _ in range(NC)]   # int16 [128, k] blocks
    rnk_cols = [[] for _ in range(NC)]   # f32 [128] columns
    icol = [0]
    rcol = [0]

    def pack_gather(base, sel_core_edges, nch):
        """Append idx columns for one dma_gather; returns icol0."""
        i0 = icol[0]
        for c in range(NC):
            es = sel_core_edges[c]
            vals = np.zeros(nch * P, np.int64)
            vals[:len(es)] = e_srcrank[es] - base
            assert vals.max(initial=0) < SEG and vals.min(initial=0) >= 0
            blk = np.zeros((128, nch * 8), np.int16)
            # HW (queue 0) reads index i at partition 16 + i%16, col i//16;
            # the interpreter reads partitions 0..15 — fill both.
            blk[:16, :] = vals.astype(np.int16).reshape(nch * 8, 16).T
            blk[16:32, :] = blk[:16, :]
            idx_cols[c].append(blk)
        icol[0] += nch * 8
        return i0

    def pack_phase(l, sel, is_fresh):
        """Pack one (level, phase); returns phase dict or None."""
        percore_all = [er_idx[sel & (e_core == c)] for c in range(NC)]
        if all(len(x) == 0 for x in percore_all):
            return None
        if is_fresh:
            # fresh: single segment based at the previous level's compacted
            # slab, which spans [L2_off[l-1], L2_off[l])
            bases = [int(L2_off[l - 1])]
            bound = int(L2_off[l])
        else:
            segs = sorted(set((e_srcrank[sel] // SEG).tolist()))
            bases = [int(s * SEG) for s in segs]
            bound = int(L2_off[l - 1])
        gathers = []
        incids = []
        ch0 = 0
        for base in bases:
            rows = min(SEG, bound - base)
            sel_b = sel & (e_srcrank >= base) & (e_srcrank < base + SEG)
            percore = []
            for c in range(NC):
                es = er_idx[sel_b & (e_core == c)]
                es = es[np.argsort(e_wl[es], kind="stable")]
                percore.append(es)
            cnt = max(len(x) for x in percore)
            if cnt == 0:
                continue
            nch = (cnt + P - 1) // P
            i0 = pack_gather(base, percore, nch)
            for j in range(nch):
                groups = set()
                chunk_es = [x[j * P:(j + 1) * P] for x in percore]
                for es in chunk_es:
                    groups.update((e_wl[es] // GW).tolist())
                for g in sorted(groups):
                    for c in range(NC):
                        es = chunk_es[c]
                        rv = np.full(P, -1.0, np.float16)
                        ing = (e_wl[es] // GW) == g
                        rv[np.where(ing)[0]] = (e_wl[es[ing]] - g * GW)
                        rnk_cols[c].append(rv)
                    incids.append([ch0 + j, int(g), rcol[0], False])
                    rcol[0] += 1
            gathers.append((base, rows, nch, i0, ch0))
            ch0 += nch
        if not gathers:
            return None
        return {"gathers": gathers, "incids": incids, "nch_total": ch0}

    er_idx = np.arange(len(er), dtype=np.int64)
    levels = [None] * NL
    for l in range(1, NL):
        in_lvl = e_lvl == l
        old = pack_phase(l, in_lvl & ~e_fresh, False)
        fresh = pack_phase(l, in_lvl & e_fresh, True)
        ngrp = (int(Vc[l]) + GW - 1) // GW
        # stop-flag per psum group: seeds -> old -> fresh (emission order)
        seed_stop = [True] * ngrp
        for ph in (old, fresh):
            if ph is None:
                continue
            for it in ph["incids"]:
                seed_stop[it[1]] = False
        lastmark = {}
        for name, ph in (("old", old), ("fresh", fresh)):
            if ph is None:
                continue
            for i, it in enumerate(ph["incids"]):
                lastmark[it[1]] = (name, i)
        for name, ph in (("old", old), ("fresh", fresh)):
            if ph is None:
                continue
            for i, it in enumerate(ph["incids"]):
                it[3] = lastmark.get(it[1]) == (name, i)
        levels[l] = {"old": old, "fresh": fresh, "seed_stop": seed_stop,
                     "ngrp": ngrp}

    TC = max(icol[0], 8)
    RC = max(rcol[0], 1)
    idxs = np.zeros((NC, 128, TC), np.int16)
    rnks = np.full((NC, 128, RC), -1.0, np.float16)
    for c in range(NC):
        if idx_cols[c]:
            idxs[c, :, :icol[0]] = np.concatenate(idx_cols[c], axis=1)
        if rnk_cols[c]:
            rnks[c, :, :rcol[0]] = np.stack(rnk_cols[c], axis=1)

    return {
        "N": N, "NL": NL, "n_l": n_l, "pad": pad_l, "L_off": L_off,
        "Vc": Vc, "Voff": Voff, "nblk": nblk, "sumVc": sumVc,
        "NpadTot": NpadTot, "TC": TC, "RC": RC,
        "Kpad": Kpad, "L2_off": L2_off, "NpadTot2": NpadTot2,
        "rank2": rank2,
        "levels": levels, "idxs": idxs, "rnks": rnks, "cnts": cnts,
        "node_of_rank": node_of_rank, "lvl0": lvl0, "constn": constn,
    }


# ---------------------------------------------------------------------------
# Bass program
# ---------------------------------------------------------------------------

def _build(sched):
    import concourse.bacc as bacc
    import concourse.tile as tile
    from concourse import bass, mybir

    f32 = mybir.dt.float32
    bf16 = mybir.dt.float16
    i16 = mybir.dt.int16
    AF = mybir.ActivationFunctionType
    OP = mybir.AluOpType
    NL = sched["NL"]
    L_off = sched["L_off"]
    Vc = sched["Vc"]
    Voff = sched["Voff"]
    pad = sched["pad"]
    TC = sched["TC"]
    RC = sched["RC"]
    sumVc = sched["sumVc"]
    NpadTot = sched["NpadTot"]
    Kpad = sched["Kpad"]
    L2_off = sched["L2_off"]
    NpadTot2 = sched["NpadTot2"]
    Vcmax = int(Vc.max())
    Kpadmax = max(int(Kpad.max()), P)
    RG = [list(range(NC))]

    nc = bacc.Bacc("TRN2", target_bir_lowering=False, debug=False,
                   enable_asserts=False, num_devices=NC)

    wmat_d = nc.dram_tensor("wmat", [P, P * len(WM)], bf16, kind="ExternalInput")
    vc_d = nc.dram_tensor("vcols", [P, len(VC)], f32, kind="ExternalInput")
    vcb_d = nc.dram_tensor("vcolsb", [P, 2], bf16, kind="ExternalInput")
    rowc_d = nc.dram_tensor("rowc", [P, 1024], f32, kind="ExternalInput")
    vr3_d = nc.dram_tensor("vr3", [3, P], bf16, kind="ExternalInput")
    cnt_d = nc.dram_tensor("cnts", [3, sumVc], bf16, kind="ExternalInput")
    idx_d = nc.dram_tensor("idxs", [P, TC], i16, kind="ExternalInput")
    rnk_d = nc.dram_tensor("rnks", [P, RC], bf16, kind="ExternalInput")
    io16_d = nc.dram_tensor("iota16", [P, 1536], bf16, kind="ExternalInput")
    pred_d = nc.dram_tensor("pred", [sumVc], f32, kind="ExternalOutput")
    h_store = nc.dram_tensor("h_store", [NpadTot2, D], bf16, kind="Internal",
                             addr_space="Shared")
    ag_in = [nc.dram_tensor(f"ag_in{i}", [Kpadmax, D], bf16, kind="Internal")
             for i in range(2)]

    with tile.TileContext(nc) as tc:
        cpool = tc.alloc_tile_pool(name="const", bufs=1)
        spool = tc.alloc_tile_pool(name="sbuf", bufs=3)
        gpool = tc.alloc_tile_pool(name="gath", bufs=2)
        opool = tc.alloc_tile_pool(name="oh", bufs=2)
        hpool = tc.alloc_tile_pool(name="hnew", bufs=6)
        ppool = tc.alloc_tile_pool(name="psS", bufs=3, space="PSUM")
        qpool = tc.alloc_tile_pool(name="psG", bufs=3, space="PSUM")
        tpool = tc.alloc_tile_pool(name="psT", bufs=1, space="PSUM")
        rpool = tc.alloc_tile_pool(name="psP", bufs=1, space="PSUM")

        # ---- load constants ----
        wm = cpool.tile([P, P * len(WM)], bf16, tag="wm")
        nc.sync.dma_start(out=wm[:], in_=wmat_d[:])
        vc = cpool.tile([P, len(VC)], f32, tag="vc")
        nc.sync.dma_start(out=vc[:], in_=vc_d[:])
        vcb = cpool.tile([P, 2], bf16, tag="vcb")
        nc.sync.dma_start(out=vcb[:], in_=vcb_d[:])
        rowc = cpool.tile([P, 1024], f32, tag="rowc")
        nc.sync.dma_start(out=rowc[:], in_=rowc_d[:])
        vr3 = cpool.tile([3, P], bf16, tag="vr3")
        nc.sync.dma_start(out=vr3[:], in_=vr3_d[:])
        idxs = cpool.tile([P, TC], i16, tag="idxs")
        nc.sync.dma_start(out=idxs[:], in_=idx_d[:])
        rnks = cpool.tile([P, RC], bf16, tag="rnks")
        nc.sync.dma_start(out=rnks[:], in_=rnk_d[:])
        io16 = cpool.tile([P, 1536], bf16, tag="io16")
        nc.sync.dma_start(out=io16[:], in_=io16_d[:])

        def wmb(name):
            return wm[:, WM[name] * P:(WM[name] + 1) * P]

        def vcc(name):
            return vc[:, VC[name]:VC[name] + 1]

        # ---- per-level state ----
        S_ps = [None] * NL
        Hg_old = [None] * (NL + 1)
        last_ag = [None]

        def grp_widths(l):
            ws = []
            v = int(Vc[l])
            while v > 0:
                ws.append(min(GW, v))
                v -= GW
            return ws

        def emit_gathers(info, which, bound_level):
            """One dma_gather per source segment; reads h_store below
            L_off[bound_level]."""
            ph = info[which]
            if ph is None:
                return None
            hg = gpool.tile([P, ph["nch_total"] * D], bf16, tag="hg_" + which)
            for (base, rows, nch, i0, ch0) in ph["gathers"]:
                gi = nc.gpsimd.dma_gather(
                    out_ap=hg[:, ch0 * D:(ch0 + nch) * D].rearrange(
                        "p (k d) -> p k d", d=D),
                    in_ap=h_store[base:base + rows, :],
                    idxs_ap=idxs[:, i0:i0 + nch * 8],
                    num_idxs=nch * P,
                    num_idxs_reg=nch * P,
                    elem_size=D,
                )
                # dynamic DRAM reads are not region-tracked by Tile's shadow
                # memory: pin the RAW edge vs the latest AllGather by hand
                if last_ag[0] is not None:
                    tile.add_dep_helper(gi.ins, last_ag[0].ins, sync=True,
                                        reason="gather reads AllGather output")
            return hg

        def emit_onehots(info, which, widths):
            ph = info[which]
            if ph is None:
                return None
            n = len(ph["incids"])
            oh = opool.tile([P, n * GW], bf16, tag="oh_" + which)
            for i, (ch, g, rc, stop) in enumerate(ph["incids"]):
                w = widths[g]
                nc.vector.tensor_tensor(
                    out=oh[:, i * GW:i * GW + w],
                    in0=rnks[:, rc:rc + 1].to_broadcast([P, w]),
                    in1=io16[:, 0:w],
                    op=OP.is_equal,
                )
            return oh

        def emit_chunks(l, which, hg, oh):
            info = sched["levels"][l]
            ph = info[which]
            if ph is None:
                return
            widths = grp_widths(l)
            for i, (ch, g, rc, stop) in enumerate(ph["incids"]):
                w = widths[g]
                nc.tensor.matmul(
                    out=S_ps[l][g][:, :w],
                    lhsT=hg[:, ch * D:(ch + 1) * D],
                    rhs=oh[:, i * GW:i * GW + w],
                    start=False, stop=stop, skip_group_check=True)

        def emit_seeds(l):
            info = sched["levels"][l]
            tiles = []
            v = int(Vc[l])
            off = int(Voff[l])
            cm = spool.tile([3, Vcmax], bf16, tag="cm")
            nc.sync.dma_start(out=cm[0:3, :v], in_=cnt_d[0:3, off:off + v])
            for g, w in enumerate(grp_widths(l)):
                sp = ppool.tile([P, GW], f32, tag="S", space="PSUM")
                nc.tensor.matmul(
                    out=sp[:, :w], lhsT=vr3[0:3, :],
                    rhs=cm[0:3, g * GW:g * GW + w],
                    start=True, stop=info["seed_stop"][g],
                    skip_group_check=True)
                tiles.append(sp)
            S_ps[l] = tiles

        def emit_mlp(l, g, w, rhs_sb):
            """MLP head for one 512-group. rhs_sb is t4 (h - h0); the W1@h0
            part lives in the b1p bias."""
            z1s = []
            for half in ("a", "b"):
                zp = qpool.tile([P, GW], f32, tag="G", space="PSUM")
                nc.tensor.matmul(out=zp[:, :w], lhsT=wmb("W1T" + half),
                                 rhs=rhs_sb[:, :w], start=True, stop=True)
                zs = spool.tile([P, GW], bf16, tag="z1" + half)
                nc.scalar.activation(out=zs[:, :w], in_=zp[:, :w],
                                     func=AF.Relu, bias=vcc("b1p" + half))
                z1s.append(zs)
            z2s = []
            for mi, mh in enumerate(("m0", "m1")):
                zp = qpool.tile([P, GW], f32, tag="G", space="PSUM")
                nc.tensor.matmul(out=zp[:, :w], lhsT=wmb("W2_k0" + mh),
                                 rhs=z1s[0][:, :w], start=True, stop=False)
                nc.tensor.matmul(out=zp[:, :w], lhsT=wmb("W2_k1" + mh),
                                 rhs=z1s[1][:, :w], start=False, stop=True)
                zs = spool.tile([P, GW], bf16, tag="z2" + mh)
                nc.scalar.activation(out=zs[:, :w], in_=zp[:, :w], func=AF.Relu,
                                     bias=vcc("b2a" if mi == 0 else "b2b"))
                z2s.append(zs)
            pp = rpool.tile([1, GW], f32, tag="pred", space="PSUM")
            nc.tensor.matmul(out=pp[:, :w], lhsT=vcb[:, 0:1], rhs=z2s[0][:, :w],
                             start=True, stop=False)
            nc.tensor.matmul(out=pp[:, :w], lhsT=vcb[:, 1:2], rhs=z2s[1][:, :w],
                             start=False, stop=True)
            ps = spool.tile([1, GW], f32, tag="psb")
            nc.scalar.activation(out=ps[:, :w], in_=pp[:, :w], func=AF.Identity,
                                 bias=vc[0:1, VC["b3"]:VC["b3"] + 1])
            off = int(Voff[l]) + g * GW
            nc.sync.dma_start(out=pred_d[off:off + w], in_=ps[0:1, :w])

        # seeds for level 1 (no old/fresh edges possible at level 1)
        emit_seeds(1)

        # ================= levels 1..NL-1 =================
        for l in range(1, NL):
            info = sched["levels"][l]
            widths = grp_widths(l)

            # fresh gather + one-hots + scatter matmuls for this level
            hg_f = emit_gathers(info, "fresh", l)
            oh_f = emit_onehots(info, "fresh", grp_widths(l))
            emit_chunks(l, "fresh", hg_f, oh_f)

            # old gather for the NEXT level: sources are below L_off[l], so
            # the dma_gather runs before this level's AllGather on the queue
            if l + 1 < NL:
                ninfo = sched["levels"][l + 1]
                Hg_old[l + 1] = (
                    emit_gathers(ninfo, "old", l),
                    emit_onehots(ninfo, "old", grp_widths(l + 1)),
                )

            # GRU per group: t4 = sigm(-z) * (tanh(n) - h0);  h = h0 + t4
            t4g = []
            for g, w in enumerate(widths):
                ssb = spool.tile([P, GW], bf16, tag="Ssb")
                nc.vector.tensor_copy(out=ssb[:, :w], in_=S_ps[l][g][:, :w])

                gr = qpool.tile([P, GW], f32, tag="G", space="PSUM")
                nc.tensor.matmul(out=gr[:, :w], lhsT=wmb("WgT_r"),
                                 rhs=ssb[:, :w], start=True, stop=True)
                gz = qpool.tile([P, GW], f32, tag="G", space="PSUM")
                nc.tensor.matmul(out=gz[:, :w], lhsT=wmb("WgT_z"),
                                 rhs=ssb[:, :w], start=True, stop=True)
                gn = qpool.tile([P, GW], f32, tag="G", space="PSUM")
                nc.tensor.matmul(out=gn[:, :w], lhsT=wmb("WgT_n"),
                                 rhs=ssb[:, :w], start=True, stop=False)

                rsb = spool.tile([P, GW], bf16, tag="rsb")
                nc.scalar.activation(out=rsb[:, :w], in_=gr[:, :w],
                                     func=AF.Sigmoid, bias=vcc("bias_r"))
                zsb = spool.tile([P, GW], f32, tag="zsb")
                nc.scalar.activation(out=zsb[:, :w], in_=gz[:, :w],
                                     func=AF.Sigmoid, bias=vcc("nbias_z"),
                                     scale=-1.0)
                nc.tensor.matmul(out=gn[:, :w], lhsT=wmb("diag_hn"),
                                 rhs=rsb[:, :w], start=False, stop=True)
                nsb = spool.tile([P, GW], f32, tag="nsb")
                nc.scalar.activation(out=nsb[:, :w], in_=gn[:, :w],
                                     func=AF.Tanh, bias=vcc("bias_n"))

                t3 = spool.tile([P, GW], f32, tag="t3")
                nc.vector.tensor_scalar(out=t3[:, :w], in0=nsb[:, :w],
                                        scalar1=vcc("h0"), scalar2=None,
                                        op0=OP.subtract)
                t4 = hpool.tile([P, GW], bf16, tag="t4")
                nc.vector.tensor_tensor(out=t4[:, :w], in0=t3[:, :w],
                                        in1=zsb[:, :w], op=OP.mult)
                t4g.append(t4)

            # transpose+stage only the source prefix (first Kpad rows of
            # this core's shard), AllGather it into the compacted h_store
            # (skipped for the last level: nothing reads it)
            kp = int(Kpad[l])
            if l < NL - 1 and kp > 0:
                agt = ag_in[l % 2]
                for g, w in enumerate(widths):
                    ws = min(w, kp - g * GW)
                    if ws <= 0:
                        break
                    tp = tpool.tile([P, GW], bf16, tag="tp", space="PSUM")
                    nb = ws // P
                    for b in range(nb):
                        nc.tensor.transpose(
                            out=tp[:, b * P:(b + 1) * P],
                            in_=t4g[g][:, b * P:(b + 1) * P],
                            identity=wmb("ident"))
                    tps = spool.tile([P, GW], bf16, tag="tps")
                    nc.vector.tensor_tensor(
                        out=tps[:, :ws], in0=tp[:, :ws],
                        in1=rowc[:, 512:512 + ws],
                        op=OP.add)
                    for b in range(nb):
                        row = g * GW + b * P
                        nc.sync.dma_start(out=agt[row:row + P, :],
                                          in_=tps[:, b * P:(b + 1) * P])
                cc = nc.gpsimd.collective_compute(
                    "AllGather", bass.mybir.AluOpType.bypass,
                    replica_groups=RG,
                    ins=[agt[0:kp, :].opt()],
                    outs=[h_store[int(L2_off[l]):int(L2_off[l]) + NC * kp,
                                  :].opt()],
                )
                last_ag[0] = cc

            # MLP head for this level (fills the AllGather latency)
            for g, w in enumerate(widths):
                emit_mlp(l, g, w, t4g[g])

            # seeds + old scatter matmuls for the next level (also fill)
            if l + 1 < NL:
                emit_seeds(l + 1)
                hg_o, oh_o = Hg_old[l + 1]
                emit_chunks(l + 1, "old", hg_o, oh_o)

        for pl in (rpool, tpool, qpool, ppool, hpool, opool, gpool, spool,
                   cpool):
            pl.release()

    nc.compile()
    return nc


# ---------------------------------------------------------------------------
# Entry point
# ---------------------------------------------------------------------------

def _run(inputs, trace=False):
    from concourse.bass_utils import run_bass_kernel_spmd

    fl = np.asarray(inputs["forward_level"])
    num_levels = int(fl.max()) + 1
    sched = _preprocess(fl, inputs["edge_index"], num_levels)
    wts = _prep_weights(inputs)
    bf16 = np.float16
    iota16 = np.tile(np.arange(1536, dtype=np.float16)[None, :], (128, 1))

    key = (sched["N"], sched["TC"], sched["RC"], sched["sumVc"],
           tuple(int(x) for x in sched["Vc"]),
           tuple((0 if lv is None else
                  (0 if lv["old"] is None else len(lv["old"]["incids"]),
                   0 if lv["fresh"] is None else len(lv["fresh"]["incids"])))
                 for lv in sched["levels"]))
    if key not in _COMPILED:
        _COMPILED[key] = _build(sched)
    nc = _COMPILED[key]

    in_maps = []
    for c in range(NC):
        in_maps.append({
            "wmat": wts["wmat"], "vcols": wts["vcols"],
            "vcolsb": wts["vcolsb"], "rowc": wts["rowc"], "vr3": wts["vr3"],
            "cnts": sched["cnts"][c].astype(bf16),
            "idxs": sched["idxs"][c],
            "rnks": sched["rnks"][c],
            "iota16": iota16,
        })

    res = run_bass_kernel_spmd(nc, in_maps, core_ids=list(range(NC)),
                               trace=trace)

    NL = sched["NL"]
    L_off, Vc, Voff = sched["L_off"], sched["Vc"], sched["Voff"]
    node_of_rank = sched["node_of_rank"]
    out = np.zeros(sched["N"], np.float32)
    out[sched["lvl0"]] = wts["pred0"]
    out[sched["constn"]] = wts["predz"]
    for c in range(NC):
        oc = res.results[c]["pred"]
        for l in range(1, NL):
            gr = int(L_off[l]) + c * int(Vc[l]) + np.arange(int(Vc[l]))
            nd = node_of_rank[gr]
            m = nd >= 0
            out[nd[m]] = oc[int(Voff[l]):int(Voff[l]) + int(Vc[l])][m]
    return out[:, None], res


def kernel(**inputs):
    out, _ = _run(inputs, trace=False)
    return out
